# revision 87
# baseline (speedup 1.0000x reference)
"""GAT encoder Bass kernel for TRN2 — v2.

Architecture: dst-sharded nodes across 8 cores; per-core edge-major
"plane-major" layout [128 node-rows, ch-plane, slot]; degree-sorted 128-node
tiles with shared (max-over-core) slot schedule, slot count UNIFORM within
each 4-tile chunk so per-tile ops batch into one instruction per chunk.
Host ships fp16 halo-expanded source features per slot (x[src]), fp16
edge_attr planes, per-node x, 1/deg, and pad counts.

Device: attention logits via tensor_scalar leaves (4x DVE mode) + fp16
tensor_tensor merge trees; self-loop logit = mean of real edge logits
(per-chunk batched reduce); a_dst broadcast add on GpSimd; leaky-relu+exp on
ACT; per-chunk denominator/aggregation reduces on DVE with a pad-slot
denominator correction (no validity plane); projection 12->128 (block-diag
W_gat fp16), ELU, MLP 128->128 (PReLU) ->32 in ch-major with fp16 PE
matmuls, double-buffered PSUM, per-chunk output DMA.
"""

import numpy as np
import concourse.bass as bass
import concourse.mybir as mybir
import concourse.tile as tile
from concourse.bass import AP

F32 = mybir.dt.float32
F16 = mybir.dt.float16
AF = mybir.ActivationFunctionType
OP = mybir.AluOpType

P = 128
NEG_SLOPE = 0.2
CH = 4  # tiles per chunk (uniform slot count within a chunk)
MSHIFT = -8.0  # logit shift before exp (cancels in softmax; avoids fp16 inf)


# ---------------------------------------------------------------------------
# Tile-framework epilogue fix: this walrus build rejects >=2 sync waits on the
# kernel-tail Drain ("Too many sync wait commands").  Strip the waits off the
# drain and re-emit them as individual sync-engine nops.
# ---------------------------------------------------------------------------
def block_split(T):
    """Chunk-aligned block boundaries shared by host layout and device."""
    NG = (T + CH - 1) // CH
    a = max(1, (NG - 3) // 2)
    ngs = [2, a, NG - 3 - a, 1]
    bts = [0]
    for n in ngs:
        bts.append(min(bts[-1] + n * CH, T))
    bts[-1] = T
    return [(bts[i], bts[i + 1]) for i in range(len(ngs))
            if bts[i] < bts[i + 1]]


def patch_tile_epilogue():
    from concourse.tile import ScopedClock
    import bass_rust

    if getattr(tile.TileContext, "_gatk_patched", False):
        return

    orig_lower = tile.TileContext._lower_ordered_insts

    def _lower_ordered_insts(self, ordered):
        for bb_name, insts in list(ordered.items()):
            out = []
            for inst in insts:
                si = inst.sync_info
                if si is not None and si.on_wait and len(si.on_wait) > 1:
                    waits = list(si.on_wait)
                    for i, w in enumerate(waits[:-1]):
                        n = bass_rust.InstNoOp(
                            name=f"{inst.name}-sw{i}", ins=[], outs=[])
                        n.engine = inst.engine
                        n.sync_info = mybir.SyncInfo(
                            on_wait=[w], on_update=[])
                        out.append(n)
                    si.on_wait.clear()
                    si.on_wait.append(waits[-1])
                out.append(inst)
            ordered[bb_name] = out
        return orig_lower(self, ordered)

    tile.TileContext._lower_ordered_insts = _lower_ordered_insts
    tile.TileContext._gatk_patched = True

    def _drain_and_barrier(self, tick_clock, wait_clock):
        drain_inst = self.nc.sync.drain()
        wait_clock.add_sem_waits(
            drain_inst.ins, ScopedClock({None: tick_clock.global_clock})
        )
        si = drain_inst.ins.sync_info
        waits = list(si.on_wait or [])
        si.on_wait.clear()
        for w in waits:
            n = self.nc.sync.nop()
            nsi = n.ins.sync_info
            if nsi is None:
                n.ins.sync_info = mybir.SyncInfo(on_wait=[w], on_update=[])
            else:
                nsi.on_wait.append(w)
        self.nc.all_engine_barrier()
        assert self.sems is not None
        popped = self.nc._tile_sem_poison_stack.pop()
        assert popped is self._sem_poison
        self.nc.clear_and_free_semaphores(list(self.sems.allocated().values()))
        self.nc.all_engine_barrier()

    tile.TileContext._drain_and_barrier = _drain_and_barrier


# ---------------------------------------------------------------------------
# Host-side sharding / layout prep (pure indexing + input redistribution).
# ---------------------------------------------------------------------------
def host_prep(x, edge_index, edge_attr, n_cores):
    N = x.shape[0]
    E = edge_index.shape[1]
    NLOC = N // n_cores
    NPAD = ((NLOC + P - 1) // P) * P
    T = NPAD // P

    src = np.asarray(edge_index[0], dtype=np.int64)
    dst = np.asarray(edge_index[1], dtype=np.int64)
    x = np.asarray(x, dtype=np.float32)
    ea = np.asarray(edge_attr, dtype=np.float32)

    deg = np.bincount(dst, minlength=N).astype(np.int64)

    # per-core degree-sorted node order
    orders = np.zeros((n_cores, NPAD), dtype=np.int64)  # sorted-pos -> local id
    ranks = np.zeros((n_cores, NPAD), dtype=np.int64)   # local id -> sorted-pos
    degp = np.zeros((n_cores, NPAD), dtype=np.int64)
    for c in range(n_cores):
        dloc = np.zeros(NPAD, dtype=np.int64)
        dloc[:NLOC] = deg[c * NLOC:(c + 1) * NLOC]
        dloc[NLOC:] = -1  # dummies first
        o = np.argsort(dloc, kind="stable")
        orders[c] = o
        ranks[c, o] = np.arange(NPAD)
        degp[c] = np.maximum(dloc[o], 0)  # sorted-pos -> degree (dummies 0)

    # shared slot schedule; D uniform within each CH-tile chunk
    D = np.zeros(T, dtype=np.int64)
    for t in range(T):
        D[t] = degp[:, t * P:(t + 1) * P].max() + 1
    for g in range((T + CH - 1) // CH):
        t0, t1 = g * CH, min((g + 1) * CH, T)
        D[t0:t1] = D[t0:t1].max()
    off = np.zeros(T + 1, dtype=np.int64)
    off[1:] = np.cumsum(D)
    S = int(off[-1])

    # edge -> (core, p, slot)
    e_core = dst // NLOC
    e_rank = ranks[e_core, dst - e_core * NLOC]
    e_t = e_rank // P
    e_p = e_rank % P
    # within-destination running index (1..deg); self-loop is slot 0
    order_e = np.argsort(dst, kind="stable")
    kk = np.empty(E, dtype=np.int64)
    ds = dst[order_e]
    grp_start = np.r_[0, np.flatnonzero(ds[1:] != ds[:-1]) + 1]
    lengths = np.diff(np.r_[grp_start, E])
    within = np.arange(E) - np.repeat(grp_start, lengths)
    kk[order_e] = within + 1
    e_s = off[e_t] + kk

    ea7 = np.zeros((n_cores, P, 7, S), dtype=np.float32)
    xgv = np.zeros((n_cores, P, 3, S), dtype=np.float32)

    ea7[e_core, e_p, :, e_s] = ea
    xgv[e_core, e_p, :, e_s] = x[src]
    bl = block_split(T)

    # self slots + per-node tables
    xn3 = np.zeros((n_cores, P, 3, T), dtype=np.float32)
    invd = np.zeros((n_cores, P, T), dtype=np.float32)
    npad = np.zeros((n_cores, P, T), dtype=np.float32)
    node_of = np.zeros((n_cores, T, P), dtype=np.int64)
    for c in range(n_cores):
        loc = orders[c]  # sorted-pos -> local id
        glob = c * NLOC + loc
        valid = loc < NLOC
        xg_nodes = np.where(valid[:, None], x[np.minimum(glob, N - 1)], 0.0)
        for t in range(T):
            sl = slice(t * P, (t + 1) * P)
            xn3[c, :, :, t] = xg_nodes[sl]
            xgv[c, :, :, off[t]] = xg_nodes[sl]
            invd[c, :, t] = 1.0 / np.maximum(degp[c, sl], 1)
            npad[c, :, t] = (D[t] - 1) - degp[c, sl]
            node_of[c, t] = glob[sl]

    # block-contiguous shipping layout: per block, planes packed contiguously
    ea7s = np.concatenate(
        [ea7[:, :, :, off[t0]:off[t1]].reshape(n_cores, P, -1)
         for (t0, t1) in bl], axis=2)
    xgvs = np.concatenate(
        [xgv[:, :, :, off[t0]:off[t1]].reshape(n_cores, P, -1)
         for (t0, t1) in bl], axis=2)

    # per-core fp32 tables packed into one tensor: xn3 | invd | npad | easum
    easum = np.add.reduceat(ea7, off[:-1], axis=3)  # [C, P, 7, T]
    tabs = np.concatenate([xn3.reshape(n_cores, P, -1), invd, npad,
                           easum.reshape(n_cores, P, -1)], axis=2)

    sched = dict(T=T, D=D, off=off, S=S, NLOC=NLOC, NPAD=NPAD, n_cores=n_cores)
    streams = dict(ea7=ea7s.astype(np.float16), xgv=xgvs.astype(np.float16),
                   tabs=np.ascontiguousarray(tabs))
    unscr = dict(node_of=node_of, valid_loc=orders < NLOC)
    return sched, streams, unscr


def host_weights(n_heads, C, W_gat, att_src, att_dst, W_edge, att_edge,
                 bias_gat, W1, b1, prelu_a, W2, b2):
    """Pure-layout reshapes/replications/casts of the weight tensors.

    packw [P, 20] fp32:  0:3 W_gat.T | 3:10 W_edge.T | 10 att_src |
      11 att_dst | 12 att_edge | 13:17 hmask | 17 bias_gat | 18 b1 |
      19 b2 (rows 0:32)
    pack16 [P, 160] fp16:  0:128 W1 | 128:160 W2
    """
    HC = n_heads * C
    nj_x = W_gat.shape[0]
    packw = np.zeros((P, 20), dtype=np.float32)
    packw[:, 0:3] = W_gat.T
    packw[:, 3:10] = W_edge.T
    packw[:, 10] = att_src.reshape(HC)
    packw[:, 11] = att_dst.reshape(HC)
    packw[:, 12] = att_edge.reshape(HC)
    for h in range(n_heads):
        packw[h * C:(h + 1) * C, 13 + h] = 1.0
    packw[:, 17] = bias_gat
    packw[:, 18] = b1
    packw[:b2.shape[0], 19] = b2
    pack16 = np.zeros((P, HC + 32), dtype=np.float16)
    pack16[:, 0:HC] = W1
    pack16[:, HC:HC + 32] = W2
    wpj = np.zeros((nj_x * n_heads, HC), dtype=np.float32)
    for h in range(n_heads):
        wpj[nj_x * h: nj_x * (h + 1), C * h: C * (h + 1)] = \
            W_gat[:, C * h: C * (h + 1)]
    return dict(
        packw=packw,
        pack16=pack16,
        wpj16=np.ascontiguousarray(wpj, dtype=np.float16),
        ident=np.eye(P, dtype=np.float32),
    )


# ---------------------------------------------------------------------------
# Device program.
# ---------------------------------------------------------------------------
def build_program(sched, n_heads=4, nj_x=3, nj_e=7, lat=32,
                  prelu_alpha=0.25):
    T = sched["T"]
    D = sched["D"]
    off = sched["off"]
    S = sched["S"]
    HC = P  # hidden dim == 128 == partitions
    H = n_heads
    NG = (T + CH - 1) // CH  # chunk groups

    nc = bass.Bass()
    dt = F32

    # --- dram I/O ---
    ea7_d = nc.dram_tensor("ea7", [P, nj_e * S], F16, kind="ExternalInput")
    xgv_d = nc.dram_tensor("xgv", [P, nj_x * S], F16, kind="ExternalInput")
    tabs_d = nc.dram_tensor("tabs", [P, 12 * T], dt, kind="ExternalInput")
    packw_d = nc.dram_tensor("packw", [P, 20], dt, kind="ExternalInput")
    pack16_d = nc.dram_tensor("pack16", [P, HC + lat], F16,
                              kind="ExternalInput")
    wpj16_d = nc.dram_tensor("wpj16", [nj_x * H, HC], F16, kind="ExternalInput")
    id_d = nc.dram_tensor("ident", [P, P], dt, kind="ExternalInput")
    # output in channel-major [lat, (t, p)]; host transposes in unscramble
    out_d = nc.dram_tensor("out", [lat, T * P], dt, kind="ExternalOutput")

    NSC = nj_e * H + nj_x * H + nj_x * H  # scale columns: V | U_src | U_dst
    OFF_V, OFF_US, OFF_UD = 0, nj_e * H, nj_e * H + nj_x * H

    # blocks split at chunk boundaries; small first block to fill the pipe,
    # small last block to shorten the un-overlapped phase-2 tail
    bl = block_split(T)
    SBmax = max(int(off[t1] - off[t0]) for (t0, t1) in bl)
    Dmax = int(D.max())
    CW = CH * P  # phase-2 chunk column width

    with tile.TileContext(nc) as tc:
        with (
            tc.tile_pool(name="wp", bufs=1) as wp,
            tc.tile_pool(name="sp", bufs=2) as sp,
            tc.tile_pool(name="mp", bufs=3) as mp,
            tc.tile_pool(name="pp", bufs=2, space="PSUM") as pp,
            tc.tile_pool(name="pq", bufs=1, space="PSUM") as pq,
        ):
            # ---------------- phase 0: weights & derived ----------------
            pw = wp.tile([P, 20], dt, tag="pw")
            tabs = wp.tile([P, 12 * T], dt, tag="tabs")
            pk16 = wp.tile([P, HC + lat], F16, tag="pk16")
            wpj = wp.tile([nj_x * H, HC], F16, tag="wpj")
            ident = wp.tile([P, P], dt, tag="ident")
            nc.sync.dma_start(pw[:], packw_d[:])
            nc.sync.dma_start(tabs[:], tabs_d[:])
            # slices of the packed tensors
            wgT = pw[:, 0:3]
            weT = pw[:, 3:10]
            asc = pw[:, 10:11]
            adc = pw[:, 11:12]
            aec = pw[:, 12:13]
            bgc = pw[:, 17:18]
            b1c = pw[:, 18:19]
            b2c = pw[0:lat, 19:20]
            w1s = pk16[:, 0:HC]
            w2s = pk16[:, HC:HC + lat]
            XNS0, IVD0, NPD0, EAS0 = 0, 3 * T, 4 * T, 5 * T
            onesr = wp.tile([1, P], dt, tag="onesr")
            nc.vector.memset(onesr[:], 1.0)
            mcol = wp.tile([P, 1], dt, tag="mcol")
            nc.vector.memset(mcol[:], MSHIFT)

            # W28 = W_edgeT (j-major x H) * head-mask ; W12 same from W_gatT
            w28 = wp.tile([HC, nj_e * H], dt, tag="w28")
            w12 = wp.tile([HC, nj_x * H], dt, tag="w12")
            pw_t, pw_o = pw[:].tensor, pw[:].offset
            pw_p = list(pw[:].ap[0])
            weT_b = AP(pw_t, pw_o + 3, [pw_p, [1, nj_e], [0, H]])
            hm_e = AP(pw_t, pw_o + 13, [pw_p, [0, nj_e], [1, H]])
            nc.vector.tensor_tensor(
                out=w28[:].rearrange("p (j h) -> p j h", j=nj_e),
                in0=weT_b, in1=hm_e, op=OP.mult)
            wgT_b = AP(pw_t, pw_o + 0, [pw_p, [1, nj_x], [0, H]])
            hm_x = AP(pw_t, pw_o + 13, [pw_p, [0, nj_x], [1, H]])
            nc.vector.tensor_tensor(
                out=w12[:].rearrange("p (j h) -> p j h", j=nj_x),
                in0=wgT_b, in1=hm_x, op=OP.mult)

            # scale rows via K=128 matmuls, then partition-broadcast
            ps1w = pp.tile([HC, CW], dt, tag="ps1")
            ps2w = pp.tile([HC, CW], dt, tag="ps2")
            srow = wp.tile([1, NSC], dt, tag="srow")
            psv = ps1w[0:1, 0:NSC]
            nc.tensor.matmul(psv[:, 0:nj_e * H], aec, w28[:],
                             start=True, stop=True)
            nc.tensor.matmul(psv[:, OFF_US:OFF_US + nj_x * H], asc, w12[:],
                             start=True, stop=True)
            nc.tensor.matmul(psv[:, OFF_UD:OFF_UD + nj_x * H], adc, w12[:],
                             start=True, stop=True)
            nc.vector.tensor_copy(srow[:], psv)
            scal = wp.tile([P, NSC], dt, tag="scal")
            psb = ps2w[0:P, 0:NSC]
            nc.tensor.matmul(psb, onesr[:], srow[:], start=True, stop=True)
            nc.vector.tensor_copy(scal[:], psb)

            ad_all = wp.tile([P, H * T], dt, tag="ad_all")
            ad16 = wp.tile([P, H * T], F16, tag="ad16")
            corr = wp.tile([P, H * T], dt, tag="corr")
            tpr = wp.tile([P, H * T], F16, tag="tpr")
            av_all = wp.tile([P, H * T], dt, tag="av_all")
            avm16 = wp.tile([P, H * T], F16, tag="avm16")
            LV = wp.tile([P, H * nj_e * SBmax], F16, tag="LV")

            def emit_ad_corr():
                # ad_all [P, H, T] from xn planes (a_dst per node)
                for h in range(H):
                    adh = ad_all[:, h * T:(h + 1) * T]
                    nc.vector.tensor_scalar(
                        out=adh, in0=tabs[:, XNS0:XNS0 + T],
                        scalar1=scal[:, OFF_UD + h: OFF_UD + h + 1],
                        scalar2=None, op0=OP.mult)
                    for j in range(1, nj_x):
                        nc.vector.scalar_tensor_tensor(
                            out=adh,
                            in0=tabs[:, XNS0 + j * T: XNS0 + (j + 1) * T],
                            scalar=scal[:, OFF_UD + j * H + h:
                                        OFF_UD + j * H + h + 1],
                            in1=adh, op0=OP.mult, op1=OP.add)
                nc.vector.tensor_copy(ad16[:], ad_all[:])
                # pad-slot denominator correction: corr = npad*exp(prelu(ad))
                nc.scalar.activation(tpr[:], ad16[:], AF.Prelu,
                                     alpha=NEG_SLOPE)
                nc.scalar.activation(corr[:], tpr[:], AF.Exp,
                                     bias=mcol[:, :1])
                npd_b = AP(tabs[:].tensor, tabs[:].offset + NPD0,
                           [list(tabs[:].ap[0]), [0, H], [1, T]])
                nc.vector.tensor_tensor(out=corr[:].rearrange(
                    "p (h t) -> p h t", h=H), in0=corr[:].rearrange(
                    "p (h t) -> p h t", h=H), in1=npd_b, op=OP.mult)
                # self-loop logit base: mean of incoming V-logits per node
                # av_all[p,h,t] = sum_j easum_j * V[j,h];  avm16 = av * invd
                for h in range(H):
                    avh = av_all[:, h * T:(h + 1) * T]
                    nc.vector.tensor_scalar(
                        out=avh, in0=tabs[:, EAS0:EAS0 + T],
                        scalar1=scal[:, OFF_V + h: OFF_V + h + 1],
                        scalar2=None, op0=OP.mult)
                    for j in range(1, nj_e):
                        nc.vector.scalar_tensor_tensor(
                            out=avh,
                            in0=tabs[:, EAS0 + j * T: EAS0 + (j + 1) * T],
                            scalar=scal[:, OFF_V + j * H + h:
                                        OFF_V + j * H + h + 1],
                            in1=avh, op0=OP.mult, op1=OP.add)
                ivd_f = AP(tabs[:].tensor, tabs[:].offset + IVD0,
                           [list(tabs[:].ap[0]), [0, H], [1, T]])
                nc.vector.tensor_tensor(out=avm16[:].rearrange(
                    "p (h t) -> p h t", h=H), in0=av_all[:].rearrange(
                    "p (h t) -> p h t", h=H), in1=ivd_f, op=OP.mult)

            # persistent per-chunk accumulators
            den_g = [wp.tile([P, H * CH], dt, tag=f"den{g}", name=f"den{g}")
                     for g in range(NG)]
            agg_g = [wp.tile([P, H * nj_x * CH], dt, tag=f"agg{g}",
                             name=f"agg{g}") for g in range(NG)]

            def chunks_of(bt0, bt1):
                return range(bt0 // CH, (bt1 + CH - 1) // CH)

            # ------------- per-block edge pipeline (sw-pipelined) ----------
            # Emission order: trees(0) | [logits-tail(b) ; trees(b+1) ;
            # post-exp(b) ; phase2(b)] so DVE never waits on pool/ACT.
            blk = {}

            def emit_dma(b):
                bt0, bt1 = bl[b]
                o0, o1 = int(off[bt0]), int(off[bt1])
                SB = o1 - o0
                # block-contiguous dram offsets
                eoff = sum(nj_e * (int(off[t1]) - int(off[t0]))
                           for (t0, t1) in bl[:b])
                xoff = sum(nj_x * (int(off[t1]) - int(off[t0]))
                           for (t0, t1) in bl[:b])
                eab = sp.tile([P, nj_e * SBmax], F16, tag="eab")
                xgb = sp.tile([P, nj_x * SBmax], F16, tag="xgb")
                aev = sp.tile([P, H * SBmax], F16, tag="aev")
                exb = sp.tile([P, H * SBmax], F16, tag="exb")
                scu = sp.tile([P, H * nj_x * SBmax], F16, tag="scu")
                blk[b] = (o0, SB, eab, xgb, aev, exb, None, scu)
                half = (nj_e * SB) // 2
                nc.sync.dma_start(eab[:, :half], ea7_d[:, eoff: eoff + half])
                nc.sync.dma_start(eab[:, half: nj_e * SB],
                                  ea7_d[:, eoff + half: eoff + nj_e * SB])
                nc.sync.dma_start(xgb[:, :nj_x * SB],
                                  xgv_d[:, xoff: xoff + nj_x * SB])

            def emit_trees(b):
                bt0, bt1 = bl[b]
                o0, SB, eab, xgb, aev, exb, scr, scu = blk[b]
                ae_t, ae_o = aev[:].tensor, aev[:].offset
                ae_p = list(aev[:].ap[0])
                M = SBmax  # uniform plane stride in LV / scu
                lv_t, lv_o = LV[:].tensor, LV[:].offset
                lv_p = list(LV[:].ap[0])
                su_t, su_o = scu[:].tensor, scu[:].offset
                su_p = list(scu[:].ap[0])
                lvs = lambda k: LV[:, k * M: k * M + SB]
                sus = lambda k: scu[:, k * M: k * M + SB]

                # U-tree leaves + V-leaves j=5,6 off-DVE (ACT + GpSimd)
                for h in range(H):
                    for j in range(2):
                        nc.scalar.activation(
                            sus(h * nj_x + j), xgb[:, j * SB:(j + 1) * SB],
                            AF.Copy,
                            scale=scal[:, OFF_US + j * H + h:
                                       OFF_US + j * H + h + 1])
                    nc.gpsimd.tensor_scalar(
                        out=sus(h * nj_x + 2), in0=xgb[:, 2 * SB:3 * SB],
                        scalar1=scal[:, OFF_US + 2 * H + h:
                                     OFF_US + 2 * H + h + 1],
                        scalar2=None, op0=OP.mult)
                for h in range(H):
                    nc.scalar.activation(
                        lvs(h * nj_e + 5), eab[:, 5 * SB:6 * SB],
                        AF.Copy,
                        scale=scal[:, OFF_V + 5 * H + h:
                                   OFF_V + 5 * H + h + 1])
                    nc.gpsimd.tensor_scalar(
                        out=lvs(h * nj_e + 6), in0=eab[:, 6 * SB:7 * SB],
                        scalar1=scal[:, OFF_V + 6 * H + h:
                                     OFF_V + 6 * H + h + 1],
                        scalar2=None, op0=OP.mult)

                # V-leaves j=0..4 on DVE
                for h in range(H):
                    for j in range(5):
                        nc.vector.tensor_scalar(
                            out=lvs(h * nj_e + j),
                            in0=eab[:, j * SB:(j + 1) * SB],
                            scalar1=scal[:, OFF_V + j * H + h:
                                         OFF_V + j * H + h + 1],
                            scalar2=None, op0=OP.mult)

                # merge tree batched across heads (7 TT instrs per block)
                ap3 = lambda t, o, p, d: AP(t, o, [p] + d)
                # B1: lv[7h+{0,2,4}] += lv[7h+{1,3,5}]
                nc.vector.tensor_tensor(
                    out=ap3(lv_t, lv_o, lv_p,
                            [[nj_e * M, H], [2 * M, 3], [1, SB]]),
                    in0=ap3(lv_t, lv_o, lv_p,
                            [[nj_e * M, H], [2 * M, 3], [1, SB]]),
                    in1=ap3(lv_t, lv_o + M, lv_p,
                            [[nj_e * M, H], [2 * M, 3], [1, SB]]),
                    op=OP.add)
                # B2: lv[7h+4] += lv[7h+6]
                nc.vector.tensor_tensor(
                    out=ap3(lv_t, lv_o + 4 * M, lv_p,
                            [[nj_e * M, H], [1, SB]]),
                    in0=ap3(lv_t, lv_o + 4 * M, lv_p,
                            [[nj_e * M, H], [1, SB]]),
                    in1=ap3(lv_t, lv_o + 6 * M, lv_p,
                            [[nj_e * M, H], [1, SB]]),
                    op=OP.add)
                # B3: lv[7h] += lv[7h+2] ; B4: lv[7h] += lv[7h+4]
                for o1 in (2 * M, 4 * M):
                    nc.vector.tensor_tensor(
                        out=ap3(lv_t, lv_o, lv_p, [[nj_e * M, H], [1, SB]]),
                        in0=ap3(lv_t, lv_o, lv_p, [[nj_e * M, H], [1, SB]]),
                        in1=ap3(lv_t, lv_o + o1, lv_p,
                                [[nj_e * M, H], [1, SB]]),
                        op=OP.add)
                # B5/B6: su[3h] += su[3h+1] ; su[3h] += su[3h+2]
                for o1 in (M, 2 * M):
                    nc.vector.tensor_tensor(
                        out=ap3(su_t, su_o, su_p, [[nj_x * M, H], [1, SB]]),
                        in0=ap3(su_t, su_o, su_p, [[nj_x * M, H], [1, SB]]),
                        in1=ap3(su_t, su_o + o1, su_p,
                                [[nj_x * M, H], [1, SB]]),
                        op=OP.add)
                # B7: ae[h] = lv[7h] + su[3h]
                nc.vector.tensor_tensor(
                    out=ap3(ae_t, ae_o, ae_p, [[SB, H], [1, SB]]),
                    in0=ap3(lv_t, lv_o, lv_p, [[nj_e * M, H], [1, SB]]),
                    in1=ap3(su_t, su_o, su_p, [[nj_x * M, H], [1, SB]]),
                    op=OP.add)

                # self-loop slot0 += mean of incoming V-logits (per chunk)
                for g in chunks_of(bt0, bt1):
                    ta, tb = g * CH, min((g + 1) * CH, T)
                    tcn = tb - ta
                    dt_g = int(D[ta])
                    lt = int(off[ta]) - o0
                    sl0 = AP(ae_t, ae_o + lt,
                             [ae_p, [dt_g, tcn], [SB, H]])
                    avm_b = AP(avm16[:].tensor, avm16[:].offset + ta,
                               [list(avm16[:].ap[0]), [1, tcn], [T, H]])
                    nc.vector.tensor_tensor(out=sl0, in0=sl0, in1=avm_b,
                                            op=OP.add)

            def emit_logits_tail(b):
                """a_dst add (GpSimd) + leaky-relu + exp (ACT)."""
                bt0, bt1 = bl[b]
                o0, SB, eab, xgb, aev, exb, scr, scu = blk[b]
                ae_t, ae_o = aev[:].tensor, aev[:].offset
                ae_p = list(aev[:].ap[0])
                for g in chunks_of(bt0, bt1):
                    ta, tb = g * CH, min((g + 1) * CH, T)
                    tcn = tb - ta
                    dt_g = int(D[ta])
                    lt = int(off[ta]) - o0
                    sl = AP(ae_t, ae_o + lt,
                            [ae_p, [dt_g, tcn], [SB, H], [1, dt_g]])
                    adb = AP(ad16[:].tensor, ad16[:].offset + ta,
                             [list(ad16[:].ap[0]), [1, tcn], [T, H], [0, dt_g]])
                    nc.vector.tensor_tensor(out=sl, in0=sl, in1=adb, op=OP.add)
                nc.scalar.activation(aev[:, :H * SB], aev[:, :H * SB],
                                     AF.Prelu, alpha=NEG_SLOPE)
                nc.scalar.activation(exb[:, :H * SB], aev[:, :H * SB], AF.Exp,
                                     bias=mcol[:, :1])

            def emit_post_chunk(b, g):
                """denominators, weighted messages, aggregation for chunk g."""
                o0, SB, eab, xgb, aev, exb, scr, scu = blk[b]
                ex_t, ex_o = exb[:].tensor, exb[:].offset
                ex_p = list(exb[:].ap[0])
                if True:
                    ta, tb = g * CH, min((g + 1) * CH, T)
                    tcn = tb - ta
                    dt_g = int(D[ta])
                    lt = int(off[ta]) - o0
                    dg = den_g[g]
                    ag = agg_g[g]

                    nc.vector.tensor_reduce(
                        out=AP(dg[:].tensor, dg[:].offset,
                               [list(dg[:].ap[0]), [1, tcn], [CH, H]]),
                        in_=AP(ex_t, ex_o + lt,
                               [ex_p, [dt_g, tcn], [SB, H], [1, dt_g]]),
                        axis=mybir.AxisListType.X, op=OP.add)
                    # den -= npad * exp(prelu(a_dst))   (pad-slot correction)
                    cor_b = AP(corr[:].tensor, corr[:].offset + ta,
                               [list(corr[:].ap[0]), [1, tcn], [T, H]])
                    dg_b = AP(dg[:].tensor, dg[:].offset,
                              [list(dg[:].ap[0]), [1, tcn], [CH, H]])
                    nc.vector.tensor_tensor(out=dg_b, in0=dg_b, in1=cor_b,
                                            op=OP.subtract)

                    msg = mp.tile([P, H * nj_x * CH * Dmax], F16, tag="msg")
                    m_t, m_o, m_p = (msg[:].tensor, msg[:].offset,
                                     list(msg[:].ap[0]))
                    nd = nj_x * dt_g
                    for ti in range(ta, tb):
                        lt_i = int(off[ta]) - o0 + (ti - ta) * dt_g
                        m_ap = AP(m_t, m_o + (ti - ta) * H * nd,
                                  [m_p, [nd, H], [dt_g, nj_x], [1, dt_g]])
                        ealpha = AP(ex_t, ex_o + lt_i,
                                    [ex_p, [SB, H], [0, nj_x], [1, dt_g]])
                        xgs = AP(xgb[:].tensor, xgb[:].offset + lt_i,
                                 [list(xgb[:].ap[0]), [0, H], [SB, nj_x],
                                  [1, dt_g]])
                        nc.vector.tensor_tensor(out=m_ap, in0=ealpha, in1=xgs,
                                                op=OP.mult)
                    nc.vector.tensor_reduce(
                        out=AP(ag[:].tensor, ag[:].offset,
                               [list(ag[:].ap[0]), [1, tcn], [CH, H * nj_x]]),
                        in_=AP(m_t, m_o,
                               [m_p, [H * nd, tcn], [dt_g, H * nj_x],
                                [1, dt_g]]),
                        axis=mybir.AxisListType.X, op=OP.add)

            def emit_phase2_chunk(g, tail=False):
                if True:
                    ta, tb = g * CH, min((g + 1) * CH, T)
                    tcn = tb - ta
                    cw = tcn * P
                    dg = den_g[g]
                    ag = agg_g[g]

                    # alpha-normalize: agg_n = agg / den (fp32)
                    rec = mp.tile([P, H * CH], dt, tag="rec")
                    nc.vector.reciprocal(rec[:], dg[:])
                    agn = mp.tile([P, H * nj_x * CH], dt, tag="agn")
                    ag_b = AP(ag[:].tensor, ag[:].offset,
                              [list(ag[:].ap[0]), [CH * nj_x, H], [CH, nj_x],
                               [1, tcn]])
                    an_b = AP(agn[:].tensor, agn[:].offset,
                              [list(agn[:].ap[0]), [CH * nj_x, H], [CH, nj_x],
                               [1, tcn]])
                    rec_b = AP(rec[:].tensor, rec[:].offset,
                               [list(rec[:].ap[0]), [CH, H], [0, nj_x],
                                [1, tcn]])
                    nc.vector.tensor_tensor(out=an_b, in0=ag_b, in1=rec_b,
                                            op=OP.mult)

                    # transpose agg_n -> [12, cw] then to fp16 for matmul
                    pst = pq.tile([nj_x * H, CW], dt, tag="pst")
                    for ti in range(ta, tb):
                        nc.tensor.transpose(
                            out=pst[:, (ti - ta) * P:(ti - ta + 1) * P],
                            in_=AP(agn[:].tensor, agn[:].offset + (ti - ta),
                                   [list(agn[:].ap[0]), [CH, nj_x * H]]),
                            identity=ident[:])
                    aggT = mp.tile([nj_x * H, CW], F16, tag="aggT")
                    if tail:
                        nc.vector.tensor_copy(aggT[:, :cw], pst[:, :cw])
                    else:
                        nc.scalar.copy(aggT[:, :cw], pst[:, :cw])

                    ps1 = pp.tile([HC, CW], dt, tag="ps1")
                    nc.tensor.matmul(ps1[:, :cw], wpj[:], aggT[:, :cw],
                                     start=True, stop=True)
                    # ELU(z+bg): min(exp(z+bg),1) - 1 + relu(z+bg)
                    r1 = mp.tile([HC, CW], F16, tag="r1")
                    u1 = mp.tile([HC, CW], F16, tag="u1")
                    nc.scalar.activation(r1[:, :cw], ps1[:, :cw], AF.Relu,
                                         bias=bgc)
                    nc.scalar.activation(u1[:, :cw], ps1[:, :cw], AF.Exp,
                                         bias=bgc)
                    nc.vector.tensor_scalar(out=u1[:, :cw], in0=u1[:, :cw],
                                            scalar1=1.0, scalar2=-1.0,
                                            op0=OP.min, op1=OP.add)
                    nc.vector.tensor_tensor(out=r1[:, :cw], in0=r1[:, :cw],
                                            in1=u1[:, :cw], op=OP.add)

                    ps2 = pp.tile([HC, CW], dt, tag="ps2")
                    nc.tensor.matmul(ps2[:, :cw], w1s[:], r1[:, :cw],
                                     start=True, stop=True)
                    h2 = mp.tile([HC, CW], F16, tag="h2")
                    nc.scalar.activation(h2[:, :cw], ps2[:, :cw], AF.Prelu,
                                         bias=b1c, alpha=prelu_alpha)

                    ps3 = pp.tile([lat, CW], dt, tag="ps3")
                    nc.tensor.matmul(ps3[:, :cw], w2s[:], h2[:, :cw],
                                     start=True, stop=True)
                    o3 = mp.tile([lat, CW], dt, tag="o3")
                    if tail:
                        nc.vector.tensor_scalar(
                            out=o3[:, :cw], in0=ps3[:, :cw],
                            scalar1=b2c, scalar2=None, op0=OP.add)
                    else:
                        nc.scalar.activation(o3[:, :cw], ps3[:, :cw],
                                             AF.Identity, bias=b2c)
                    nc.sync.dma_start(out_d[:, ta * P: ta * P + cw],
                                      o3[:, :cw])

            NB = len(bl)
            for b in range(min(2, NB)):
                emit_dma(b)
            # phase-2-only weights: issue after the first input blocks
            nc.sync.dma_start(pk16[:], pack16_d[:])
            nc.sync.dma_start(wpj[:], wpj16_d[:])
            nc.sync.dma_start(ident[:], id_d[:])
            emit_ad_corr()
            emit_trees(0)
            p2q = []  # chunks with post emitted, phase2 pending (lag 2)
            for b in range(NB):
                emit_logits_tail(b)
                if b + 2 < NB:
                    emit_dma(b + 2)
                if b + 1 < NB:
                    emit_trees(b + 1)
                for g in chunks_of(*bl[b]):
                    emit_post_chunk(b, g)
                    p2q.append(g)
                    if len(p2q) > 3:
                        emit_phase2_chunk(p2q.pop(0), tail=(b == NB - 1))
            for g in p2q:
                emit_phase2_chunk(g, tail=True)

    return nc


# ---------------------------------------------------------------------------
# Full kernel entry (host orchestration).
# ---------------------------------------------------------------------------
def make_in_maps(sched, streams, w, n_cores):
    maps = []
    for c in range(n_cores):
        m = dict(
            ea7=streams["ea7"][c].reshape(P, -1),
            xgv=streams["xgv"][c].reshape(P, -1),
            tabs=streams["tabs"][c],
        )
        m.update(w)
        maps.append(m)
    return maps


def unscramble(results, sched, unscr, N, lat=32):
    n_cores = sched["n_cores"]
    T = sched["T"]
    out = np.zeros((N, lat), dtype=np.float32)
    for c in range(n_cores):
        o = results[c]["out"].reshape(lat, T, P).transpose(2, 1, 0)
        node_of = unscr["node_of"][c]  # [T, P] global ids (clamped for dummies)
        valid = unscr["valid_loc"][c].reshape(T, P)
        for t in range(T):
            v = valid[t]
            out[node_of[t][v]] = o[v, t]
    return out


# ---------------------------------------------------------------------------
# Self-contained harness entry: kernel(**inputs) -> full [N, 32] output.
# ---------------------------------------------------------------------------
_CACHE = {}


def kernel(x, edge_index, edge_attr, W_gat, att_src, att_dst, W_edge,
           att_edge, bias_gat, W1, b1, prelu_a, W2, b2):
    from concourse.bass_utils import run_bass_kernel_spmd

    patch_tile_epilogue()
    n_cores = 8
    x = np.asarray(x)
    edge_index = np.asarray(edge_index)
    edge_attr = np.asarray(edge_attr)
    H, C = np.asarray(att_src).shape

    sched, streams, unscr = host_prep(x, edge_index, edge_attr, n_cores)
    w = host_weights(H, C, np.asarray(W_gat), np.asarray(att_src),
                     np.asarray(att_dst), np.asarray(W_edge),
                     np.asarray(att_edge), np.asarray(bias_gat),
                     np.asarray(W1), np.asarray(b1), np.asarray(prelu_a),
                     np.asarray(W2), np.asarray(b2))

    key = (sched["T"], sched["S"], tuple(int(d) for d in sched["D"]),
           float(np.asarray(prelu_a)))
    if key not in _CACHE:
        _CACHE[key] = build_program(sched, n_heads=H,
                                    prelu_alpha=float(np.asarray(prelu_a)))
    nc = _CACHE[key]

    maps = make_in_maps(sched, streams, w, n_cores)
    res = run_bass_kernel_spmd(nc, maps, core_ids=list(range(n_cores)))
    out = unscramble(res.results, sched, unscr, x.shape[0])
    return out.astype(np.float32)


# revision 89
# speedup vs baseline: 2.4208x; 2.4208x over previous
"""GAT encoder Bass kernel for TRN2 — v2.

Architecture: dst-sharded nodes across 8 cores; per-core edge-major
"plane-major" layout [128 node-rows, ch-plane, slot]; degree-sorted 128-node
tiles with shared (max-over-core) slot schedule, slot count UNIFORM within
each 4-tile chunk so per-tile ops batch into one instruction per chunk.
Host ships fp16 halo-expanded source features per slot (x[src]), fp16
edge_attr planes, per-node x, 1/deg, and pad counts.

Device: attention logits via tensor_scalar leaves (4x DVE mode) + fp16
tensor_tensor merge trees; self-loop logit = mean of real edge logits
(per-chunk batched reduce); a_dst broadcast add on GpSimd; leaky-relu+exp on
ACT; per-chunk denominator/aggregation reduces on DVE with a pad-slot
denominator correction (no validity plane); projection 12->128 (block-diag
W_gat fp16), ELU, MLP 128->128 (PReLU) ->32 in ch-major with fp16 PE
matmuls, double-buffered PSUM, per-chunk output DMA.
"""

import numpy as np
import concourse.bass as bass
import concourse.mybir as mybir
import concourse.tile as tile
from concourse.bass import AP

F32 = mybir.dt.float32
F16 = mybir.dt.float16
AF = mybir.ActivationFunctionType
OP = mybir.AluOpType

P = 128
NEG_SLOPE = 0.2
CH = 4  # tiles per chunk (uniform slot count within a chunk)
MSHIFT = -8.0  # logit shift before exp (cancels in softmax; avoids fp16 inf)


# ---------------------------------------------------------------------------
# Tile-framework epilogue fix: this walrus build rejects >=2 sync waits on the
# kernel-tail Drain ("Too many sync wait commands").  Strip the waits off the
# drain and re-emit them as individual sync-engine nops.
# ---------------------------------------------------------------------------
def block_split(T):
    """Chunk-aligned block boundaries shared by host layout and device."""
    NG = (T + CH - 1) // CH
    a = max(1, (NG - 3) // 2)
    ngs = [2, a, NG - 3 - a, 1]
    bts = [0]
    for n in ngs:
        bts.append(min(bts[-1] + n * CH, T))
    bts[-1] = T
    return [(bts[i], bts[i + 1]) for i in range(len(ngs))
            if bts[i] < bts[i + 1]]


def patch_tile_epilogue():
    from concourse.tile import ScopedClock
    import bass_rust

    if getattr(tile.TileContext, "_gatk_patched", False):
        return

    orig_lower = tile.TileContext._lower_ordered_insts

    def _lower_ordered_insts(self, ordered):
        for bb_name, insts in list(ordered.items()):
            out = []
            for inst in insts:
                si = inst.sync_info
                if si is not None and si.on_wait and len(si.on_wait) > 1:
                    waits = list(si.on_wait)
                    for i, w in enumerate(waits[:-1]):
                        n = bass_rust.InstNoOp(
                            name=f"{inst.name}-sw{i}", ins=[], outs=[])
                        n.engine = inst.engine
                        n.sync_info = mybir.SyncInfo(
                            on_wait=[w], on_update=[])
                        out.append(n)
                    si.on_wait.clear()
                    si.on_wait.append(waits[-1])
                out.append(inst)
            ordered[bb_name] = out
        return orig_lower(self, ordered)

    tile.TileContext._lower_ordered_insts = _lower_ordered_insts
    tile.TileContext._gatk_patched = True

    def _drain_and_barrier(self, tick_clock, wait_clock):
        drain_inst = self.nc.sync.drain()
        wait_clock.add_sem_waits(
            drain_inst.ins, ScopedClock({None: tick_clock.global_clock})
        )
        si = drain_inst.ins.sync_info
        waits = list(si.on_wait or [])
        si.on_wait.clear()
        for w in waits:
            n = self.nc.sync.nop()
            nsi = n.ins.sync_info
            if nsi is None:
                n.ins.sync_info = mybir.SyncInfo(on_wait=[w], on_update=[])
            else:
                nsi.on_wait.append(w)
        self.nc.all_engine_barrier()
        assert self.sems is not None
        popped = self.nc._tile_sem_poison_stack.pop()
        assert popped is self._sem_poison
        self.nc.clear_and_free_semaphores(list(self.sems.allocated().values()))
        self.nc.all_engine_barrier()

    tile.TileContext._drain_and_barrier = _drain_and_barrier


# ---------------------------------------------------------------------------
# Host-side sharding / layout prep (pure indexing + input redistribution).
# ---------------------------------------------------------------------------
def host_prep(x, edge_index, edge_attr, n_cores):
    N = x.shape[0]
    E = edge_index.shape[1]
    NLOC = N // n_cores
    NPAD = ((NLOC + P - 1) // P) * P
    T = NPAD // P

    src = np.asarray(edge_index[0], dtype=np.int64)
    dst = np.asarray(edge_index[1], dtype=np.int64)
    x = np.asarray(x, dtype=np.float32)
    ea = np.asarray(edge_attr, dtype=np.float32)

    deg = np.bincount(dst, minlength=N).astype(np.int64)

    # per-core degree-sorted node order
    orders = np.zeros((n_cores, NPAD), dtype=np.int64)  # sorted-pos -> local id
    ranks = np.zeros((n_cores, NPAD), dtype=np.int64)   # local id -> sorted-pos
    degp = np.zeros((n_cores, NPAD), dtype=np.int64)
    for c in range(n_cores):
        dloc = np.zeros(NPAD, dtype=np.int64)
        dloc[:NLOC] = deg[c * NLOC:(c + 1) * NLOC]
        dloc[NLOC:] = -1  # dummies first
        o = np.argsort(dloc, kind="stable")
        orders[c] = o
        ranks[c, o] = np.arange(NPAD)
        degp[c] = np.maximum(dloc[o], 0)  # sorted-pos -> degree (dummies 0)

    # shared slot schedule; D uniform within each CH-tile chunk
    D = np.zeros(T, dtype=np.int64)
    for t in range(T):
        D[t] = degp[:, t * P:(t + 1) * P].max() + 1
    for g in range((T + CH - 1) // CH):
        t0, t1 = g * CH, min((g + 1) * CH, T)
        D[t0:t1] = D[t0:t1].max()
    off = np.zeros(T + 1, dtype=np.int64)
    off[1:] = np.cumsum(D)
    S = int(off[-1])

    # edge -> (core, p, slot)
    e_core = dst // NLOC
    e_rank = ranks[e_core, dst - e_core * NLOC]
    e_t = e_rank // P
    e_p = e_rank % P
    # within-destination running index (1..deg); self-loop is slot 0
    order_e = np.argsort(dst, kind="stable")
    kk = np.empty(E, dtype=np.int64)
    ds = dst[order_e]
    grp_start = np.r_[0, np.flatnonzero(ds[1:] != ds[:-1]) + 1]
    lengths = np.diff(np.r_[grp_start, E])
    within = np.arange(E) - np.repeat(grp_start, lengths)
    kk[order_e] = within + 1
    e_s = off[e_t] + kk

    ea7 = np.zeros((n_cores, P, 7, S), dtype=np.float32)
    xgv = np.zeros((n_cores, P, 3, S), dtype=np.float32)

    ea7[e_core, e_p, :, e_s] = ea
    xgv[e_core, e_p, :, e_s] = x[src]
    bl = block_split(T)

    # self slots + per-node tables
    xn3 = np.zeros((n_cores, P, 3, T), dtype=np.float32)
    invd = np.zeros((n_cores, P, T), dtype=np.float32)
    npad = np.zeros((n_cores, P, T), dtype=np.float32)
    node_of = np.zeros((n_cores, T, P), dtype=np.int64)
    for c in range(n_cores):
        loc = orders[c]  # sorted-pos -> local id
        glob = c * NLOC + loc
        valid = loc < NLOC
        xg_nodes = np.where(valid[:, None], x[np.minimum(glob, N - 1)], 0.0)
        for t in range(T):
            sl = slice(t * P, (t + 1) * P)
            xn3[c, :, :, t] = xg_nodes[sl]
            xgv[c, :, :, off[t]] = xg_nodes[sl]
            invd[c, :, t] = 1.0 / np.maximum(degp[c, sl], 1)
            npad[c, :, t] = (D[t] - 1) - degp[c, sl]
            node_of[c, t] = glob[sl]

    # block-contiguous shipping layout: per block, planes packed contiguously
    ea7s = np.concatenate(
        [ea7[:, :, :, off[t0]:off[t1]].reshape(n_cores, P, -1)
         for (t0, t1) in bl], axis=2)
    xgvs = np.concatenate(
        [xgv[:, :, :, off[t0]:off[t1]].reshape(n_cores, P, -1)
         for (t0, t1) in bl], axis=2)

    # per-core fp32 tables packed into one tensor: xn3 | invd | npad | easum
    easum = np.add.reduceat(ea7, off[:-1], axis=3)  # [C, P, 7, T]
    tabs = np.concatenate([xn3.reshape(n_cores, P, -1), invd, npad,
                           easum.reshape(n_cores, P, -1)], axis=2)

    sched = dict(T=T, D=D, off=off, S=S, NLOC=NLOC, NPAD=NPAD, n_cores=n_cores)
    streams = dict(ea7=ea7s.astype(np.float16), xgv=xgvs.astype(np.float16),
                   tabs=np.ascontiguousarray(tabs))
    unscr = dict(node_of=node_of, valid_loc=orders < NLOC)
    return sched, streams, unscr


def host_weights(n_heads, C, W_gat, att_src, att_dst, W_edge, att_edge,
                 bias_gat, W1, b1, prelu_a, W2, b2):
    """Pure-layout reshapes/replications/casts of the weight tensors.

    packw [P, 20] fp32:  0:3 W_gat.T | 3:10 W_edge.T | 10 att_src |
      11 att_dst | 12 att_edge | 13:17 hmask | 17 bias_gat | 18 b1 |
      19 b2 (rows 0:32)
    pack16 [P, 160] fp16:  0:128 W1 | 128:160 W2
    """
    HC = n_heads * C
    nj_x = W_gat.shape[0]
    packw = np.zeros((P, 20), dtype=np.float32)
    packw[:, 0:3] = W_gat.T
    packw[:, 3:10] = W_edge.T
    packw[:, 10] = att_src.reshape(HC)
    packw[:, 11] = att_dst.reshape(HC)
    packw[:, 12] = att_edge.reshape(HC)
    for h in range(n_heads):
        packw[h * C:(h + 1) * C, 13 + h] = 1.0
    packw[:, 17] = bias_gat
    packw[:, 18] = b1
    packw[:b2.shape[0], 19] = b2
    pack16 = np.zeros((P, HC + 32), dtype=np.float16)
    pack16[:, 0:HC] = W1
    pack16[:, HC:HC + 32] = W2
    wpj = np.zeros((nj_x * n_heads, HC), dtype=np.float32)
    for h in range(n_heads):
        wpj[nj_x * h: nj_x * (h + 1), C * h: C * (h + 1)] = \
            W_gat[:, C * h: C * (h + 1)]
    return dict(
        packw=packw,
        pack16=pack16,
        wpj16=np.ascontiguousarray(wpj, dtype=np.float16),
        ident=np.eye(P, dtype=np.float32),
    )


# ---------------------------------------------------------------------------
# Device program.
# ---------------------------------------------------------------------------
def build_program(sched, n_heads=4, nj_x=3, nj_e=7, lat=32,
                  prelu_alpha=0.25):
    T = sched["T"]
    D = sched["D"]
    off = sched["off"]
    S = sched["S"]
    HC = P  # hidden dim == 128 == partitions
    H = n_heads
    NG = (T + CH - 1) // CH  # chunk groups

    nc = bass.Bass()
    dt = F32

    # --- dram I/O ---
    ea7_d = nc.dram_tensor("ea7", [P, nj_e * S], F16, kind="ExternalInput")
    xgv_d = nc.dram_tensor("xgv", [P, nj_x * S], F16, kind="ExternalInput")
    tabs_d = nc.dram_tensor("tabs", [P, 12 * T], dt, kind="ExternalInput")
    packw_d = nc.dram_tensor("packw", [P, 20], dt, kind="ExternalInput")
    pack16_d = nc.dram_tensor("pack16", [P, HC + lat], F16,
                              kind="ExternalInput")
    wpj16_d = nc.dram_tensor("wpj16", [nj_x * H, HC], F16, kind="ExternalInput")
    id_d = nc.dram_tensor("ident", [P, P], dt, kind="ExternalInput")
    # output in channel-major [lat, (t, p)]; host transposes in unscramble
    out_d = nc.dram_tensor("out", [lat, T * P], dt, kind="ExternalOutput")

    NSC = nj_e * H + nj_x * H + nj_x * H  # scale columns: V | U_src | U_dst
    OFF_V, OFF_US, OFF_UD = 0, nj_e * H, nj_e * H + nj_x * H

    # blocks split at chunk boundaries; small first block to fill the pipe,
    # small last block to shorten the un-overlapped phase-2 tail
    bl = block_split(T)
    SBmax = max(int(off[t1] - off[t0]) for (t0, t1) in bl)
    Dmax = int(D.max())
    CW = CH * P  # phase-2 chunk column width

    with tile.TileContext(nc) as tc:
        with (
            tc.tile_pool(name="wp", bufs=1) as wp,
            tc.tile_pool(name="sp", bufs=2) as sp,
            tc.tile_pool(name="mp", bufs=3) as mp,
            tc.tile_pool(name="pp", bufs=2, space="PSUM") as pp,
            tc.tile_pool(name="pq", bufs=1, space="PSUM") as pq,
        ):
            # ---------------- phase 0: weights & derived ----------------
            pw = wp.tile([P, 20], dt, tag="pw")
            tabs = wp.tile([P, 12 * T], dt, tag="tabs")
            pk16 = wp.tile([P, HC + lat], F16, tag="pk16")
            wpj = wp.tile([nj_x * H, HC], F16, tag="wpj")
            ident = wp.tile([P, P], dt, tag="ident")
            nc.sync.dma_start(pw[:], packw_d[:])
            nc.sync.dma_start(tabs[:], tabs_d[:])
            # slices of the packed tensors
            wgT = pw[:, 0:3]
            weT = pw[:, 3:10]
            asc = pw[:, 10:11]
            adc = pw[:, 11:12]
            aec = pw[:, 12:13]
            bgc = pw[:, 17:18]
            b1c = pw[:, 18:19]
            b2c = pw[0:lat, 19:20]
            w1s = pk16[:, 0:HC]
            w2s = pk16[:, HC:HC + lat]
            XNS0, IVD0, NPD0, EAS0 = 0, 3 * T, 4 * T, 5 * T
            onesr = wp.tile([1, P], dt, tag="onesr")
            nc.vector.memset(onesr[:], 1.0)
            mcol = wp.tile([P, 1], dt, tag="mcol")
            nc.vector.memset(mcol[:], MSHIFT)

            # W28 = W_edgeT (j-major x H) * head-mask ; W12 same from W_gatT
            w28 = wp.tile([HC, nj_e * H], dt, tag="w28")
            w12 = wp.tile([HC, nj_x * H], dt, tag="w12")
            pw_t, pw_o = pw[:].tensor, pw[:].offset
            pw_p = list(pw[:].ap[0])
            weT_b = AP(pw_t, pw_o + 3, [pw_p, [1, nj_e], [0, H]])
            hm_e = AP(pw_t, pw_o + 13, [pw_p, [0, nj_e], [1, H]])
            nc.vector.tensor_tensor(
                out=w28[:].rearrange("p (j h) -> p j h", j=nj_e),
                in0=weT_b, in1=hm_e, op=OP.mult)
            wgT_b = AP(pw_t, pw_o + 0, [pw_p, [1, nj_x], [0, H]])
            hm_x = AP(pw_t, pw_o + 13, [pw_p, [0, nj_x], [1, H]])
            nc.vector.tensor_tensor(
                out=w12[:].rearrange("p (j h) -> p j h", j=nj_x),
                in0=wgT_b, in1=hm_x, op=OP.mult)

            # scale rows via K=128 matmuls, then partition-broadcast
            ps1w = pp.tile([HC, CW], dt, tag="ps1")
            ps2w = pp.tile([HC, CW], dt, tag="ps2")
            srow = wp.tile([1, NSC], dt, tag="srow")
            psv = ps1w[0:1, 0:NSC]
            nc.tensor.matmul(psv[:, 0:nj_e * H], aec, w28[:],
                             start=True, stop=True)
            nc.tensor.matmul(psv[:, OFF_US:OFF_US + nj_x * H], asc, w12[:],
                             start=True, stop=True)
            nc.tensor.matmul(psv[:, OFF_UD:OFF_UD + nj_x * H], adc, w12[:],
                             start=True, stop=True)
            nc.vector.tensor_copy(srow[:], psv)
            scal = wp.tile([P, NSC], dt, tag="scal")
            psb = ps2w[0:P, 0:NSC]
            nc.tensor.matmul(psb, onesr[:], srow[:], start=True, stop=True)
            nc.vector.tensor_copy(scal[:], psb)

            ad_all = wp.tile([P, H * T], dt, tag="ad_all")
            ad16 = wp.tile([P, H * T], F16, tag="ad16")
            corr = wp.tile([P, H * T], dt, tag="corr")
            tpr = wp.tile([P, H * T], F16, tag="tpr")
            av_all = wp.tile([P, H * T], dt, tag="av_all")
            avm16 = wp.tile([P, H * T], F16, tag="avm16")
            LV = wp.tile([P, H * nj_e * SBmax], F16, tag="LV")

            def emit_ad_corr():
                # ad_all [P, H, T] from xn planes (a_dst per node)
                for h in range(H):
                    adh = ad_all[:, h * T:(h + 1) * T]
                    nc.vector.tensor_scalar(
                        out=adh, in0=tabs[:, XNS0:XNS0 + T],
                        scalar1=scal[:, OFF_UD + h: OFF_UD + h + 1],
                        scalar2=None, op0=OP.mult)
                    for j in range(1, nj_x):
                        nc.vector.scalar_tensor_tensor(
                            out=adh,
                            in0=tabs[:, XNS0 + j * T: XNS0 + (j + 1) * T],
                            scalar=scal[:, OFF_UD + j * H + h:
                                        OFF_UD + j * H + h + 1],
                            in1=adh, op0=OP.mult, op1=OP.add)
                nc.vector.tensor_copy(ad16[:], ad_all[:])
                # pad-slot denominator correction: corr = npad*exp(prelu(ad))
                nc.scalar.activation(tpr[:], ad16[:], AF.Prelu,
                                     alpha=NEG_SLOPE)
                nc.scalar.activation(corr[:], tpr[:], AF.Exp,
                                     bias=mcol[:, :1])
                npd_b = AP(tabs[:].tensor, tabs[:].offset + NPD0,
                           [list(tabs[:].ap[0]), [0, H], [1, T]])
                nc.vector.tensor_tensor(out=corr[:].rearrange(
                    "p (h t) -> p h t", h=H), in0=corr[:].rearrange(
                    "p (h t) -> p h t", h=H), in1=npd_b, op=OP.mult)
                # self-loop logit base: mean of incoming V-logits per node
                # av_all[p,h,t] = sum_j easum_j * V[j,h];  avm16 = av * invd
                for h in range(H):
                    avh = av_all[:, h * T:(h + 1) * T]
                    nc.vector.tensor_scalar(
                        out=avh, in0=tabs[:, EAS0:EAS0 + T],
                        scalar1=scal[:, OFF_V + h: OFF_V + h + 1],
                        scalar2=None, op0=OP.mult)
                    for j in range(1, nj_e):
                        nc.vector.scalar_tensor_tensor(
                            out=avh,
                            in0=tabs[:, EAS0 + j * T: EAS0 + (j + 1) * T],
                            scalar=scal[:, OFF_V + j * H + h:
                                        OFF_V + j * H + h + 1],
                            in1=avh, op0=OP.mult, op1=OP.add)
                ivd_f = AP(tabs[:].tensor, tabs[:].offset + IVD0,
                           [list(tabs[:].ap[0]), [0, H], [1, T]])
                nc.vector.tensor_tensor(out=avm16[:].rearrange(
                    "p (h t) -> p h t", h=H), in0=av_all[:].rearrange(
                    "p (h t) -> p h t", h=H), in1=ivd_f, op=OP.mult)

            # persistent per-chunk accumulators
            den_g = [wp.tile([P, H * CH], dt, tag=f"den{g}", name=f"den{g}")
                     for g in range(NG)]
            agg_g = [wp.tile([P, H * nj_x * CH], dt, tag=f"agg{g}",
                             name=f"agg{g}") for g in range(NG)]

            def chunks_of(bt0, bt1):
                return range(bt0 // CH, (bt1 + CH - 1) // CH)

            # ------------- per-block edge pipeline (sw-pipelined) ----------
            # Emission order: trees(0) | [logits-tail(b) ; trees(b+1) ;
            # post-exp(b) ; phase2(b)] so DVE never waits on pool/ACT.
            blk = {}

            def emit_dma(b):
                bt0, bt1 = bl[b]
                o0, o1 = int(off[bt0]), int(off[bt1])
                SB = o1 - o0
                # block-contiguous dram offsets
                eoff = sum(nj_e * (int(off[t1]) - int(off[t0]))
                           for (t0, t1) in bl[:b])
                xoff = sum(nj_x * (int(off[t1]) - int(off[t0]))
                           for (t0, t1) in bl[:b])
                eab = sp.tile([P, nj_e * SBmax], F16, tag="eab")
                xgb = sp.tile([P, nj_x * SBmax], F16, tag="xgb")
                aev = sp.tile([P, H * SBmax], F16, tag="aev")
                exb = sp.tile([P, H * SBmax], F16, tag="exb")
                scu = sp.tile([P, H * nj_x * SBmax], F16, tag="scu")
                blk[b] = (o0, SB, eab, xgb, aev, exb, None, scu)
                half = (nj_e * SB) // 2
                nc.sync.dma_start(eab[:, :half], ea7_d[:, eoff: eoff + half])
                nc.sync.dma_start(eab[:, half: nj_e * SB],
                                  ea7_d[:, eoff + half: eoff + nj_e * SB])
                nc.sync.dma_start(xgb[:, :nj_x * SB],
                                  xgv_d[:, xoff: xoff + nj_x * SB])

            def emit_trees(b):
                bt0, bt1 = bl[b]
                o0, SB, eab, xgb, aev, exb, scr, scu = blk[b]
                ae_t, ae_o = aev[:].tensor, aev[:].offset
                ae_p = list(aev[:].ap[0])
                M = SBmax  # uniform plane stride in LV / scu
                lv_t, lv_o = LV[:].tensor, LV[:].offset
                lv_p = list(LV[:].ap[0])
                su_t, su_o = scu[:].tensor, scu[:].offset
                su_p = list(scu[:].ap[0])
                lvs = lambda k: LV[:, k * M: k * M + SB]
                sus = lambda k: scu[:, k * M: k * M + SB]

                # U-tree leaves + V-leaves j=5,6 on ACT — overlap DVE leaves
                for h in range(H):
                    for j in range(nj_x):
                        nc.scalar.activation(
                            sus(h * nj_x + j), xgb[:, j * SB:(j + 1) * SB],
                            AF.Copy,
                            scale=scal[:, OFF_US + j * H + h:
                                       OFF_US + j * H + h + 1])
                for h in range(H):
                    for j in (5, 6):
                        nc.scalar.activation(
                            lvs(h * nj_e + j), eab[:, j * SB:(j + 1) * SB],
                            AF.Copy,
                            scale=scal[:, OFF_V + j * H + h:
                                       OFF_V + j * H + h + 1])

                # V-leaves j=0..4 on DVE
                for h in range(H):
                    for j in range(5):
                        nc.vector.tensor_scalar(
                            out=lvs(h * nj_e + j),
                            in0=eab[:, j * SB:(j + 1) * SB],
                            scalar1=scal[:, OFF_V + j * H + h:
                                         OFF_V + j * H + h + 1],
                            scalar2=None, op0=OP.mult)

                # merge tree batched across heads (7 TT instrs per block)
                ap3 = lambda t, o, p, d: AP(t, o, [p] + d)
                # B1: lv[7h+{0,2,4}] += lv[7h+{1,3,5}]
                nc.vector.tensor_tensor(
                    out=ap3(lv_t, lv_o, lv_p,
                            [[nj_e * M, H], [2 * M, 3], [1, SB]]),
                    in0=ap3(lv_t, lv_o, lv_p,
                            [[nj_e * M, H], [2 * M, 3], [1, SB]]),
                    in1=ap3(lv_t, lv_o + M, lv_p,
                            [[nj_e * M, H], [2 * M, 3], [1, SB]]),
                    op=OP.add)
                # B2: lv[7h+4] += lv[7h+6]
                nc.vector.tensor_tensor(
                    out=ap3(lv_t, lv_o + 4 * M, lv_p,
                            [[nj_e * M, H], [1, SB]]),
                    in0=ap3(lv_t, lv_o + 4 * M, lv_p,
                            [[nj_e * M, H], [1, SB]]),
                    in1=ap3(lv_t, lv_o + 6 * M, lv_p,
                            [[nj_e * M, H], [1, SB]]),
                    op=OP.add)
                # B3: lv[7h] += lv[7h+2] ; B4: lv[7h] += lv[7h+4]
                for o1 in (2 * M, 4 * M):
                    nc.vector.tensor_tensor(
                        out=ap3(lv_t, lv_o, lv_p, [[nj_e * M, H], [1, SB]]),
                        in0=ap3(lv_t, lv_o, lv_p, [[nj_e * M, H], [1, SB]]),
                        in1=ap3(lv_t, lv_o + o1, lv_p,
                                [[nj_e * M, H], [1, SB]]),
                        op=OP.add)
                # B5/B6: su[3h] += su[3h+1] ; su[3h] += su[3h+2]
                for o1 in (M, 2 * M):
                    nc.vector.tensor_tensor(
                        out=ap3(su_t, su_o, su_p, [[nj_x * M, H], [1, SB]]),
                        in0=ap3(su_t, su_o, su_p, [[nj_x * M, H], [1, SB]]),
                        in1=ap3(su_t, su_o + o1, su_p,
                                [[nj_x * M, H], [1, SB]]),
                        op=OP.add)
                # B7: ae[h] = lv[7h] + su[3h]
                nc.vector.tensor_tensor(
                    out=ap3(ae_t, ae_o, ae_p, [[SB, H], [1, SB]]),
                    in0=ap3(lv_t, lv_o, lv_p, [[nj_e * M, H], [1, SB]]),
                    in1=ap3(su_t, su_o, su_p, [[nj_x * M, H], [1, SB]]),
                    op=OP.add)

                # self-loop slot0 += mean of incoming V-logits (per chunk)
                for g in chunks_of(bt0, bt1):
                    ta, tb = g * CH, min((g + 1) * CH, T)
                    tcn = tb - ta
                    dt_g = int(D[ta])
                    lt = int(off[ta]) - o0
                    sl0 = AP(ae_t, ae_o + lt,
                             [ae_p, [dt_g, tcn], [SB, H]])
                    avm_b = AP(avm16[:].tensor, avm16[:].offset + ta,
                               [list(avm16[:].ap[0]), [1, tcn], [T, H]])
                    nc.vector.tensor_tensor(out=sl0, in0=sl0, in1=avm_b,
                                            op=OP.add)

            def emit_logits_tail(b):
                """a_dst add (GpSimd) + leaky-relu + exp (ACT)."""
                bt0, bt1 = bl[b]
                o0, SB, eab, xgb, aev, exb, scr, scu = blk[b]
                ae_t, ae_o = aev[:].tensor, aev[:].offset
                ae_p = list(aev[:].ap[0])
                for g in chunks_of(bt0, bt1):
                    ta, tb = g * CH, min((g + 1) * CH, T)
                    tcn = tb - ta
                    dt_g = int(D[ta])
                    lt = int(off[ta]) - o0
                    sl = AP(ae_t, ae_o + lt,
                            [ae_p, [dt_g, tcn], [SB, H], [1, dt_g]])
                    adb = AP(ad16[:].tensor, ad16[:].offset + ta,
                             [list(ad16[:].ap[0]), [1, tcn], [T, H], [0, dt_g]])
                    nc.vector.tensor_tensor(out=sl, in0=sl, in1=adb, op=OP.add)
                nc.scalar.activation(aev[:, :H * SB], aev[:, :H * SB],
                                     AF.Prelu, alpha=NEG_SLOPE)
                nc.scalar.activation(exb[:, :H * SB], aev[:, :H * SB], AF.Exp,
                                     bias=mcol[:, :1])

            def emit_post_chunk(b, g):
                """denominators, weighted messages, aggregation for chunk g."""
                o0, SB, eab, xgb, aev, exb, scr, scu = blk[b]
                ex_t, ex_o = exb[:].tensor, exb[:].offset
                ex_p = list(exb[:].ap[0])
                if True:
                    ta, tb = g * CH, min((g + 1) * CH, T)
                    tcn = tb - ta
                    dt_g = int(D[ta])
                    lt = int(off[ta]) - o0
                    dg = den_g[g]
                    ag = agg_g[g]

                    nc.vector.tensor_reduce(
                        out=AP(dg[:].tensor, dg[:].offset,
                               [list(dg[:].ap[0]), [1, tcn], [CH, H]]),
                        in_=AP(ex_t, ex_o + lt,
                               [ex_p, [dt_g, tcn], [SB, H], [1, dt_g]]),
                        axis=mybir.AxisListType.X, op=OP.add)
                    # den -= npad * exp(prelu(a_dst))   (pad-slot correction)
                    cor_b = AP(corr[:].tensor, corr[:].offset + ta,
                               [list(corr[:].ap[0]), [1, tcn], [T, H]])
                    dg_b = AP(dg[:].tensor, dg[:].offset,
                              [list(dg[:].ap[0]), [1, tcn], [CH, H]])
                    nc.vector.tensor_tensor(out=dg_b, in0=dg_b, in1=cor_b,
                                            op=OP.subtract)

                    msg = mp.tile([P, H * nj_x * CH * Dmax], F16, tag="msg")
                    m_t, m_o, m_p = (msg[:].tensor, msg[:].offset,
                                     list(msg[:].ap[0]))
                    nd = nj_x * dt_g
                    for ti in range(ta, tb):
                        lt_i = int(off[ta]) - o0 + (ti - ta) * dt_g
                        m_ap = AP(m_t, m_o + (ti - ta) * H * nd,
                                  [m_p, [nd, H], [dt_g, nj_x], [1, dt_g]])
                        ealpha = AP(ex_t, ex_o + lt_i,
                                    [ex_p, [SB, H], [0, nj_x], [1, dt_g]])
                        xgs = AP(xgb[:].tensor, xgb[:].offset + lt_i,
                                 [list(xgb[:].ap[0]), [0, H], [SB, nj_x],
                                  [1, dt_g]])
                        nc.vector.tensor_tensor(out=m_ap, in0=ealpha, in1=xgs,
                                                op=OP.mult)
                    nc.vector.tensor_reduce(
                        out=AP(ag[:].tensor, ag[:].offset,
                               [list(ag[:].ap[0]), [1, tcn], [CH, H * nj_x]]),
                        in_=AP(m_t, m_o,
                               [m_p, [H * nd, tcn], [dt_g, H * nj_x],
                                [1, dt_g]]),
                        axis=mybir.AxisListType.X, op=OP.add)

            def emit_phase2_chunk(g, tail=False):
                if True:
                    ta, tb = g * CH, min((g + 1) * CH, T)
                    tcn = tb - ta
                    cw = tcn * P
                    dg = den_g[g]
                    ag = agg_g[g]

                    # alpha-normalize: agg_n = agg / den (fp32)
                    rec = mp.tile([P, H * CH], dt, tag="rec")
                    nc.vector.reciprocal(rec[:], dg[:])
                    agn = mp.tile([P, H * nj_x * CH], dt, tag="agn")
                    ag_b = AP(ag[:].tensor, ag[:].offset,
                              [list(ag[:].ap[0]), [CH * nj_x, H], [CH, nj_x],
                               [1, tcn]])
                    an_b = AP(agn[:].tensor, agn[:].offset,
                              [list(agn[:].ap[0]), [CH * nj_x, H], [CH, nj_x],
                               [1, tcn]])
                    rec_b = AP(rec[:].tensor, rec[:].offset,
                               [list(rec[:].ap[0]), [CH, H], [0, nj_x],
                                [1, tcn]])
                    nc.vector.tensor_tensor(out=an_b, in0=ag_b, in1=rec_b,
                                            op=OP.mult)

                    # transpose agg_n -> [12, cw] then to fp16 for matmul
                    pst = pq.tile([nj_x * H, CW], dt, tag="pst")
                    for ti in range(ta, tb):
                        nc.tensor.transpose(
                            out=pst[:, (ti - ta) * P:(ti - ta + 1) * P],
                            in_=AP(agn[:].tensor, agn[:].offset + (ti - ta),
                                   [list(agn[:].ap[0]), [CH, nj_x * H]]),
                            identity=ident[:])
                    aggT = mp.tile([nj_x * H, CW], F16, tag="aggT")
                    if tail:
                        nc.vector.tensor_copy(aggT[:, :cw], pst[:, :cw])
                    else:
                        nc.scalar.copy(aggT[:, :cw], pst[:, :cw])

                    ps1 = pp.tile([HC, CW], dt, tag="ps1")
                    nc.tensor.matmul(ps1[:, :cw], wpj[:], aggT[:, :cw],
                                     start=True, stop=True)
                    # ELU(z+bg): min(exp(z+bg),1) - 1 + relu(z+bg)
                    r1 = mp.tile([HC, CW], F16, tag="r1")
                    u1 = mp.tile([HC, CW], F16, tag="u1")
                    nc.scalar.activation(r1[:, :cw], ps1[:, :cw], AF.Relu,
                                         bias=bgc)
                    nc.scalar.activation(u1[:, :cw], ps1[:, :cw], AF.Exp,
                                         bias=bgc)
                    nc.vector.tensor_scalar(out=u1[:, :cw], in0=u1[:, :cw],
                                            scalar1=1.0, scalar2=-1.0,
                                            op0=OP.min, op1=OP.add)
                    nc.vector.tensor_tensor(out=r1[:, :cw], in0=r1[:, :cw],
                                            in1=u1[:, :cw], op=OP.add)

                    ps2 = pp.tile([HC, CW], dt, tag="ps2")
                    nc.tensor.matmul(ps2[:, :cw], w1s[:], r1[:, :cw],
                                     start=True, stop=True)
                    h2 = mp.tile([HC, CW], F16, tag="h2")
                    nc.scalar.activation(h2[:, :cw], ps2[:, :cw], AF.Prelu,
                                         bias=b1c, alpha=prelu_alpha)

                    ps3 = pp.tile([lat, CW], dt, tag="ps3")
                    nc.tensor.matmul(ps3[:, :cw], w2s[:], h2[:, :cw],
                                     start=True, stop=True)
                    o3 = mp.tile([lat, CW], dt, tag="o3")
                    if tail:
                        nc.vector.tensor_scalar(
                            out=o3[:, :cw], in0=ps3[:, :cw],
                            scalar1=b2c, scalar2=None, op0=OP.add)
                    else:
                        nc.scalar.activation(o3[:, :cw], ps3[:, :cw],
                                             AF.Identity, bias=b2c)
                    nc.sync.dma_start(out_d[:, ta * P: ta * P + cw],
                                      o3[:, :cw])

            NB = len(bl)
            for b in range(min(2, NB)):
                emit_dma(b)
            # phase-2-only weights: issue after the first input blocks
            nc.sync.dma_start(pk16[:], pack16_d[:])
            nc.sync.dma_start(wpj[:], wpj16_d[:])
            nc.sync.dma_start(ident[:], id_d[:])
            emit_ad_corr()
            emit_trees(0)
            p2q = []  # chunks with post emitted, phase2 pending (lag 2)
            for b in range(NB):
                emit_logits_tail(b)
                if b + 2 < NB:
                    emit_dma(b + 2)
                if b + 1 < NB:
                    emit_trees(b + 1)
                lag = 1 if b == NB - 1 else 3
                for g in chunks_of(*bl[b]):
                    emit_post_chunk(b, g)
                    p2q.append(g)
                    while len(p2q) > lag:
                        emit_phase2_chunk(p2q.pop(0), tail=(b >= NB - 2))
            for g in p2q:
                emit_phase2_chunk(g, tail=True)

    return nc


# ---------------------------------------------------------------------------
# Full kernel entry (host orchestration).
# ---------------------------------------------------------------------------
def make_in_maps(sched, streams, w, n_cores):
    maps = []
    for c in range(n_cores):
        m = dict(
            ea7=streams["ea7"][c].reshape(P, -1),
            xgv=streams["xgv"][c].reshape(P, -1),
            tabs=streams["tabs"][c],
        )
        m.update(w)
        maps.append(m)
    return maps


def unscramble(results, sched, unscr, N, lat=32):
    n_cores = sched["n_cores"]
    T = sched["T"]
    out = np.zeros((N, lat), dtype=np.float32)
    for c in range(n_cores):
        o = results[c]["out"].reshape(lat, T, P).transpose(2, 1, 0)
        node_of = unscr["node_of"][c]  # [T, P] global ids (clamped for dummies)
        valid = unscr["valid_loc"][c].reshape(T, P)
        for t in range(T):
            v = valid[t]
            out[node_of[t][v]] = o[v, t]
    return out


# ---------------------------------------------------------------------------
# Self-contained harness entry: kernel(**inputs) -> full [N, 32] output.
# ---------------------------------------------------------------------------
_CACHE = {}


def kernel(x, edge_index, edge_attr, W_gat, att_src, att_dst, W_edge,
           att_edge, bias_gat, W1, b1, prelu_a, W2, b2):
    from concourse.bass_utils import run_bass_kernel_spmd

    patch_tile_epilogue()
    n_cores = 8
    x = np.asarray(x)
    edge_index = np.asarray(edge_index)
    edge_attr = np.asarray(edge_attr)
    H, C = np.asarray(att_src).shape

    sched, streams, unscr = host_prep(x, edge_index, edge_attr, n_cores)
    w = host_weights(H, C, np.asarray(W_gat), np.asarray(att_src),
                     np.asarray(att_dst), np.asarray(W_edge),
                     np.asarray(att_edge), np.asarray(bias_gat),
                     np.asarray(W1), np.asarray(b1), np.asarray(prelu_a),
                     np.asarray(W2), np.asarray(b2))

    key = (sched["T"], sched["S"], tuple(int(d) for d in sched["D"]),
           float(np.asarray(prelu_a)))
    if key not in _CACHE:
        _CACHE[key] = build_program(sched, n_heads=H,
                                    prelu_alpha=float(np.asarray(prelu_a)))
    nc = _CACHE[key]

    maps = make_in_maps(sched, streams, w, n_cores)
    res = run_bass_kernel_spmd(nc, maps, core_ids=list(range(n_cores)))
    out = unscramble(res.results, sched, unscr, x.shape[0])
    return out.astype(np.float32)


# revision 90
# speedup vs baseline: 2.4481x; 1.0113x over previous
"""GAT encoder Bass kernel for TRN2 — v2.

Architecture: dst-sharded nodes across 8 cores; per-core edge-major
"plane-major" layout [128 node-rows, ch-plane, slot]; degree-sorted 128-node
tiles with shared (max-over-core) slot schedule, slot count UNIFORM within
each 4-tile chunk so per-tile ops batch into one instruction per chunk.
Host ships fp16 halo-expanded source features per slot (x[src]), fp16
edge_attr planes, per-node x, 1/deg, and pad counts.

Device: attention logits via tensor_scalar leaves (4x DVE mode) + fp16
tensor_tensor merge trees; self-loop logit = mean of real edge logits
(per-chunk batched reduce); a_dst broadcast add on GpSimd; leaky-relu+exp on
ACT; per-chunk denominator/aggregation reduces on DVE with a pad-slot
denominator correction (no validity plane); projection 12->128 (block-diag
W_gat fp16), ELU, MLP 128->128 (PReLU) ->32 in ch-major with fp16 PE
matmuls, double-buffered PSUM, per-chunk output DMA.
"""

import numpy as np
import concourse.bass as bass
import concourse.mybir as mybir
import concourse.tile as tile
from concourse.bass import AP

F32 = mybir.dt.float32
F16 = mybir.dt.float16
AF = mybir.ActivationFunctionType
OP = mybir.AluOpType

P = 128
NEG_SLOPE = 0.2
CH = 4  # tiles per chunk (uniform slot count within a chunk)
MSHIFT = -8.0  # logit shift before exp (cancels in softmax; avoids fp16 inf)


# ---------------------------------------------------------------------------
# Tile-framework epilogue fix: this walrus build rejects >=2 sync waits on the
# kernel-tail Drain ("Too many sync wait commands").  Strip the waits off the
# drain and re-emit them as individual sync-engine nops.
# ---------------------------------------------------------------------------
def block_split(T):
    """Chunk-aligned block boundaries shared by host layout and device."""
    NG = (T + CH - 1) // CH
    a = max(1, (NG - 3) // 2)
    ngs = [2, a, NG - 3 - a, 1]
    bts = [0]
    for n in ngs:
        bts.append(min(bts[-1] + n * CH, T))
    bts[-1] = T
    return [(bts[i], bts[i + 1]) for i in range(len(ngs))
            if bts[i] < bts[i + 1]]


def patch_tile_epilogue():
    from concourse.tile import ScopedClock
    import bass_rust

    if getattr(tile.TileContext, "_gatk_patched", False):
        return

    orig_lower = tile.TileContext._lower_ordered_insts

    def _lower_ordered_insts(self, ordered):
        for bb_name, insts in list(ordered.items()):
            out = []
            for inst in insts:
                si = inst.sync_info
                if si is not None and si.on_wait and len(si.on_wait) > 1:
                    waits = list(si.on_wait)
                    for i, w in enumerate(waits[:-1]):
                        n = bass_rust.InstNoOp(
                            name=f"{inst.name}-sw{i}", ins=[], outs=[])
                        n.engine = inst.engine
                        n.sync_info = mybir.SyncInfo(
                            on_wait=[w], on_update=[])
                        out.append(n)
                    si.on_wait.clear()
                    si.on_wait.append(waits[-1])
                out.append(inst)
            ordered[bb_name] = out
        return orig_lower(self, ordered)

    tile.TileContext._lower_ordered_insts = _lower_ordered_insts
    tile.TileContext._gatk_patched = True

    def _drain_and_barrier(self, tick_clock, wait_clock):
        drain_inst = self.nc.sync.drain()
        wait_clock.add_sem_waits(
            drain_inst.ins, ScopedClock({None: tick_clock.global_clock})
        )
        si = drain_inst.ins.sync_info
        waits = list(si.on_wait or [])
        si.on_wait.clear()
        for w in waits:
            n = self.nc.sync.nop()
            nsi = n.ins.sync_info
            if nsi is None:
                n.ins.sync_info = mybir.SyncInfo(on_wait=[w], on_update=[])
            else:
                nsi.on_wait.append(w)
        self.nc.all_engine_barrier()
        assert self.sems is not None
        popped = self.nc._tile_sem_poison_stack.pop()
        assert popped is self._sem_poison
        self.nc.clear_and_free_semaphores(list(self.sems.allocated().values()))
        self.nc.all_engine_barrier()

    tile.TileContext._drain_and_barrier = _drain_and_barrier


# ---------------------------------------------------------------------------
# Host-side sharding / layout prep (pure indexing + input redistribution).
# ---------------------------------------------------------------------------
def host_prep(x, edge_index, edge_attr, n_cores):
    N = x.shape[0]
    E = edge_index.shape[1]
    NLOC = N // n_cores
    NPAD = ((NLOC + P - 1) // P) * P
    T = NPAD // P

    src = np.asarray(edge_index[0], dtype=np.int64)
    dst = np.asarray(edge_index[1], dtype=np.int64)
    x = np.asarray(x, dtype=np.float32)
    ea = np.asarray(edge_attr, dtype=np.float32)

    deg = np.bincount(dst, minlength=N).astype(np.int64)

    # per-core degree-sorted node order
    orders = np.zeros((n_cores, NPAD), dtype=np.int64)  # sorted-pos -> local id
    ranks = np.zeros((n_cores, NPAD), dtype=np.int64)   # local id -> sorted-pos
    degp = np.zeros((n_cores, NPAD), dtype=np.int64)
    for c in range(n_cores):
        dloc = np.zeros(NPAD, dtype=np.int64)
        dloc[:NLOC] = deg[c * NLOC:(c + 1) * NLOC]
        dloc[NLOC:] = -1  # dummies first
        o = np.argsort(dloc, kind="stable")
        orders[c] = o
        ranks[c, o] = np.arange(NPAD)
        degp[c] = np.maximum(dloc[o], 0)  # sorted-pos -> degree (dummies 0)

    # shared slot schedule; D uniform within each CH-tile chunk
    D = np.zeros(T, dtype=np.int64)
    for t in range(T):
        D[t] = degp[:, t * P:(t + 1) * P].max() + 1
    for g in range((T + CH - 1) // CH):
        t0, t1 = g * CH, min((g + 1) * CH, T)
        D[t0:t1] = D[t0:t1].max()
    off = np.zeros(T + 1, dtype=np.int64)
    off[1:] = np.cumsum(D)
    S = int(off[-1])

    # edge -> (core, p, slot)
    e_core = dst // NLOC
    e_rank = ranks[e_core, dst - e_core * NLOC]
    e_t = e_rank // P
    e_p = e_rank % P
    # within-destination running index (1..deg); self-loop is slot 0
    order_e = np.argsort(dst, kind="stable")
    kk = np.empty(E, dtype=np.int64)
    ds = dst[order_e]
    grp_start = np.r_[0, np.flatnonzero(ds[1:] != ds[:-1]) + 1]
    lengths = np.diff(np.r_[grp_start, E])
    within = np.arange(E) - np.repeat(grp_start, lengths)
    kk[order_e] = within + 1
    e_s = off[e_t] + kk

    ea7 = np.zeros((n_cores, P, 7, S), dtype=np.float32)
    xgv = np.zeros((n_cores, P, 3, S), dtype=np.float32)

    ea7[e_core, e_p, :, e_s] = ea
    xgv[e_core, e_p, :, e_s] = x[src]
    bl = block_split(T)

    # self slots + per-node tables
    xn3 = np.zeros((n_cores, P, 3, T), dtype=np.float32)
    invd = np.zeros((n_cores, P, T), dtype=np.float32)
    npad = np.zeros((n_cores, P, T), dtype=np.float32)
    node_of = np.zeros((n_cores, T, P), dtype=np.int64)
    for c in range(n_cores):
        loc = orders[c]  # sorted-pos -> local id
        glob = c * NLOC + loc
        valid = loc < NLOC
        xg_nodes = np.where(valid[:, None], x[np.minimum(glob, N - 1)], 0.0)
        for t in range(T):
            sl = slice(t * P, (t + 1) * P)
            xn3[c, :, :, t] = xg_nodes[sl]
            xgv[c, :, :, off[t]] = xg_nodes[sl]
            invd[c, :, t] = 1.0 / np.maximum(degp[c, sl], 1)
            npad[c, :, t] = (D[t] - 1) - degp[c, sl]
            node_of[c, t] = glob[sl]

    # block-contiguous shipping layout: per block, planes packed contiguously
    ea7s = np.concatenate(
        [ea7[:, :, :, off[t0]:off[t1]].reshape(n_cores, P, -1)
         for (t0, t1) in bl], axis=2)
    xgvs = np.concatenate(
        [xgv[:, :, :, off[t0]:off[t1]].reshape(n_cores, P, -1)
         for (t0, t1) in bl], axis=2)

    # per-core fp32 tables packed into one tensor: xn3 | invd | npad | easum
    easum = np.add.reduceat(ea7, off[:-1], axis=3)  # [C, P, 7, T]
    tabs = np.concatenate([xn3.reshape(n_cores, P, -1), invd, npad,
                           easum.reshape(n_cores, P, -1)], axis=2)

    sched = dict(T=T, D=D, off=off, S=S, NLOC=NLOC, NPAD=NPAD, n_cores=n_cores)
    streams = dict(ea7=ea7s.astype(np.float16), xgv=xgvs.astype(np.float16),
                   tabs=np.ascontiguousarray(tabs))
    unscr = dict(node_of=node_of, valid_loc=orders < NLOC)
    return sched, streams, unscr


def host_weights(n_heads, C, W_gat, att_src, att_dst, W_edge, att_edge,
                 bias_gat, W1, b1, prelu_a, W2, b2):
    """Pure-layout reshapes/replications/casts of the weight tensors.

    packw [P, 20] fp32:  0:3 W_gat.T | 3:10 W_edge.T | 10 att_src |
      11 att_dst | 12 att_edge | 13:17 hmask | 17 bias_gat | 18 b1 |
      19 b2 (rows 0:32)
    pack16 [P, 160] fp16:  0:128 W1 | 128:160 W2
    """
    HC = n_heads * C
    nj_x = W_gat.shape[0]
    packw = np.zeros((P, 20), dtype=np.float32)
    packw[:, 0:3] = W_gat.T
    packw[:, 3:10] = W_edge.T
    packw[:, 10] = att_src.reshape(HC)
    packw[:, 11] = att_dst.reshape(HC)
    packw[:, 12] = att_edge.reshape(HC)
    for h in range(n_heads):
        packw[h * C:(h + 1) * C, 13 + h] = 1.0
    packw[:, 17] = bias_gat
    packw[:, 18] = b1
    packw[:b2.shape[0], 19] = b2
    pack16 = np.zeros((P, HC + 32), dtype=np.float16)
    pack16[:, 0:HC] = W1
    pack16[:, HC:HC + 32] = W2
    wpj = np.zeros((nj_x * n_heads, HC), dtype=np.float32)
    for h in range(n_heads):
        wpj[nj_x * h: nj_x * (h + 1), C * h: C * (h + 1)] = \
            W_gat[:, C * h: C * (h + 1)]
    return dict(
        packw=packw,
        pack16=pack16,
        wpj16=np.ascontiguousarray(wpj, dtype=np.float16),
        ident=np.eye(P, dtype=np.float32),
    )


# ---------------------------------------------------------------------------
# Device program.
# ---------------------------------------------------------------------------
def build_program(sched, n_heads=4, nj_x=3, nj_e=7, lat=32,
                  prelu_alpha=0.25):
    T = sched["T"]
    D = sched["D"]
    off = sched["off"]
    S = sched["S"]
    HC = P  # hidden dim == 128 == partitions
    H = n_heads
    NG = (T + CH - 1) // CH  # chunk groups

    nc = bass.Bass()
    dt = F32

    # --- dram I/O ---
    ea7_d = nc.dram_tensor("ea7", [P, nj_e * S], F16, kind="ExternalInput")
    xgv_d = nc.dram_tensor("xgv", [P, nj_x * S], F16, kind="ExternalInput")
    tabs_d = nc.dram_tensor("tabs", [P, 12 * T], dt, kind="ExternalInput")
    packw_d = nc.dram_tensor("packw", [P, 20], dt, kind="ExternalInput")
    pack16_d = nc.dram_tensor("pack16", [P, HC + lat], F16,
                              kind="ExternalInput")
    wpj16_d = nc.dram_tensor("wpj16", [nj_x * H, HC], F16, kind="ExternalInput")
    id_d = nc.dram_tensor("ident", [P, P], dt, kind="ExternalInput")
    # output in channel-major [lat, (t, p)]; host transposes in unscramble
    out_d = nc.dram_tensor("out", [lat, T * P], dt, kind="ExternalOutput")

    NSC = nj_e * H + nj_x * H + nj_x * H  # scale columns: V | U_src | U_dst
    OFF_V, OFF_US, OFF_UD = 0, nj_e * H, nj_e * H + nj_x * H

    # blocks split at chunk boundaries; small first block to fill the pipe,
    # small last block to shorten the un-overlapped phase-2 tail
    bl = block_split(T)
    SBmax = max(int(off[t1] - off[t0]) for (t0, t1) in bl)
    Dmax = int(D.max())
    CW = CH * P  # phase-2 chunk column width

    with tile.TileContext(nc) as tc:
        with (
            tc.tile_pool(name="wp", bufs=1) as wp,
            tc.tile_pool(name="sp", bufs=2) as sp,
            tc.tile_pool(name="mp", bufs=3) as mp,
            tc.tile_pool(name="pp", bufs=2, space="PSUM") as pp,
            tc.tile_pool(name="pq", bufs=1, space="PSUM") as pq,
        ):
            # ---------------- phase 0: weights & derived ----------------
            pw = wp.tile([P, 20], dt, tag="pw")
            tabs = wp.tile([P, 12 * T], dt, tag="tabs")
            pk16 = wp.tile([P, HC + lat], F16, tag="pk16")
            wpj = wp.tile([nj_x * H, HC], F16, tag="wpj")
            ident = wp.tile([P, P], dt, tag="ident")
            nc.sync.dma_start(pw[:], packw_d[:])
            nc.sync.dma_start(tabs[:], tabs_d[:])
            # slices of the packed tensors
            wgT = pw[:, 0:3]
            weT = pw[:, 3:10]
            asc = pw[:, 10:11]
            adc = pw[:, 11:12]
            aec = pw[:, 12:13]
            bgc = pw[:, 17:18]
            b1c = pw[:, 18:19]
            b2c = pw[0:lat, 19:20]
            w1s = pk16[:, 0:HC]
            w2s = pk16[:, HC:HC + lat]
            XNS0, IVD0, NPD0, EAS0 = 0, 3 * T, 4 * T, 5 * T
            onesr = wp.tile([1, P], dt, tag="onesr")
            nc.vector.memset(onesr[:], 1.0)
            mcol = wp.tile([P, 1], dt, tag="mcol")
            nc.vector.memset(mcol[:], MSHIFT)

            # W28 = W_edgeT (j-major x H) * head-mask ; W12 same from W_gatT
            w28 = wp.tile([HC, nj_e * H], dt, tag="w28")
            w12 = wp.tile([HC, nj_x * H], dt, tag="w12")
            pw_t, pw_o = pw[:].tensor, pw[:].offset
            pw_p = list(pw[:].ap[0])
            weT_b = AP(pw_t, pw_o + 3, [pw_p, [1, nj_e], [0, H]])
            hm_e = AP(pw_t, pw_o + 13, [pw_p, [0, nj_e], [1, H]])
            nc.vector.tensor_tensor(
                out=w28[:].rearrange("p (j h) -> p j h", j=nj_e),
                in0=weT_b, in1=hm_e, op=OP.mult)
            wgT_b = AP(pw_t, pw_o + 0, [pw_p, [1, nj_x], [0, H]])
            hm_x = AP(pw_t, pw_o + 13, [pw_p, [0, nj_x], [1, H]])
            nc.vector.tensor_tensor(
                out=w12[:].rearrange("p (j h) -> p j h", j=nj_x),
                in0=wgT_b, in1=hm_x, op=OP.mult)

            # scale rows via K=128 matmuls, then partition-broadcast
            ps1w = pp.tile([HC, CW], dt, tag="ps1")
            ps2w = pp.tile([HC, CW], dt, tag="ps2")
            srow = wp.tile([1, NSC], dt, tag="srow")
            psv = ps1w[0:1, 0:NSC]
            nc.tensor.matmul(psv[:, 0:nj_e * H], aec, w28[:],
                             start=True, stop=True)
            nc.tensor.matmul(psv[:, OFF_US:OFF_US + nj_x * H], asc, w12[:],
                             start=True, stop=True)
            nc.tensor.matmul(psv[:, OFF_UD:OFF_UD + nj_x * H], adc, w12[:],
                             start=True, stop=True)
            nc.vector.tensor_copy(srow[:], psv)
            scal = wp.tile([P, NSC], dt, tag="scal")
            psb = ps2w[0:P, 0:NSC]
            nc.tensor.matmul(psb, onesr[:], srow[:], start=True, stop=True)
            nc.vector.tensor_copy(scal[:], psb)

            ad_all = wp.tile([P, H * T], dt, tag="ad_all")
            ad16 = wp.tile([P, H * T], F16, tag="ad16")
            corr = wp.tile([P, H * T], dt, tag="corr")
            tpr = wp.tile([P, H * T], F16, tag="tpr")
            av_all = wp.tile([P, H * T], dt, tag="av_all")
            avm16 = wp.tile([P, H * T], F16, tag="avm16")
            LV = wp.tile([P, H * nj_e * SBmax], F16, tag="LV")

            def emit_ad_corr():
                # ad_all [P, H, T] from xn planes (a_dst per node)
                for h in range(H):
                    adh = ad_all[:, h * T:(h + 1) * T]
                    nc.vector.tensor_scalar(
                        out=adh, in0=tabs[:, XNS0:XNS0 + T],
                        scalar1=scal[:, OFF_UD + h: OFF_UD + h + 1],
                        scalar2=None, op0=OP.mult)
                    for j in range(1, nj_x):
                        nc.vector.scalar_tensor_tensor(
                            out=adh,
                            in0=tabs[:, XNS0 + j * T: XNS0 + (j + 1) * T],
                            scalar=scal[:, OFF_UD + j * H + h:
                                        OFF_UD + j * H + h + 1],
                            in1=adh, op0=OP.mult, op1=OP.add)
                nc.vector.tensor_copy(ad16[:], ad_all[:])
                # pad-slot denominator correction: corr = npad*exp(prelu(ad))
                nc.scalar.activation(tpr[:], ad16[:], AF.Prelu,
                                     alpha=NEG_SLOPE)
                nc.scalar.activation(corr[:], tpr[:], AF.Exp,
                                     bias=mcol[:, :1])
                npd_b = AP(tabs[:].tensor, tabs[:].offset + NPD0,
                           [list(tabs[:].ap[0]), [0, H], [1, T]])
                nc.vector.tensor_tensor(out=corr[:].rearrange(
                    "p (h t) -> p h t", h=H), in0=corr[:].rearrange(
                    "p (h t) -> p h t", h=H), in1=npd_b, op=OP.mult)
                # self-loop logit base: mean of incoming V-logits per node
                # av_all[p,h,t] = sum_j easum_j * V[j,h];  avm16 = av * invd
                for h in range(H):
                    avh = av_all[:, h * T:(h + 1) * T]
                    nc.vector.tensor_scalar(
                        out=avh, in0=tabs[:, EAS0:EAS0 + T],
                        scalar1=scal[:, OFF_V + h: OFF_V + h + 1],
                        scalar2=None, op0=OP.mult)
                    for j in range(1, nj_e):
                        nc.vector.scalar_tensor_tensor(
                            out=avh,
                            in0=tabs[:, EAS0 + j * T: EAS0 + (j + 1) * T],
                            scalar=scal[:, OFF_V + j * H + h:
                                        OFF_V + j * H + h + 1],
                            in1=avh, op0=OP.mult, op1=OP.add)
                ivd_f = AP(tabs[:].tensor, tabs[:].offset + IVD0,
                           [list(tabs[:].ap[0]), [0, H], [1, T]])
                nc.vector.tensor_tensor(out=avm16[:].rearrange(
                    "p (h t) -> p h t", h=H), in0=av_all[:].rearrange(
                    "p (h t) -> p h t", h=H), in1=ivd_f, op=OP.mult)

            # persistent per-chunk accumulators
            den_g = [wp.tile([P, H * CH], dt, tag=f"den{g}", name=f"den{g}")
                     for g in range(NG)]
            agg_g = [wp.tile([P, H * nj_x * CH], dt, tag=f"agg{g}",
                             name=f"agg{g}") for g in range(NG)]

            def chunks_of(bt0, bt1):
                return range(bt0 // CH, (bt1 + CH - 1) // CH)

            # ------------- per-block edge pipeline (sw-pipelined) ----------
            # Emission order: trees(0) | [logits-tail(b) ; trees(b+1) ;
            # post-exp(b) ; phase2(b)] so DVE never waits on pool/ACT.
            blk = {}

            def emit_dma(b):
                bt0, bt1 = bl[b]
                o0, o1 = int(off[bt0]), int(off[bt1])
                SB = o1 - o0
                # block-contiguous dram offsets
                eoff = sum(nj_e * (int(off[t1]) - int(off[t0]))
                           for (t0, t1) in bl[:b])
                xoff = sum(nj_x * (int(off[t1]) - int(off[t0]))
                           for (t0, t1) in bl[:b])
                eab = sp.tile([P, nj_e * SBmax], F16, tag="eab")
                xgb = sp.tile([P, nj_x * SBmax], F16, tag="xgb")
                aev = sp.tile([P, H * SBmax], F16, tag="aev")
                exb = sp.tile([P, H * SBmax], F16, tag="exb")
                scu = sp.tile([P, H * nj_x * SBmax], F16, tag="scu")
                blk[b] = (o0, SB, eab, xgb, aev, exb, None, scu)
                half = (nj_e * SB) // 2
                nc.sync.dma_start(eab[:, :half], ea7_d[:, eoff: eoff + half])
                nc.sync.dma_start(eab[:, half: nj_e * SB],
                                  ea7_d[:, eoff + half: eoff + nj_e * SB])
                nc.sync.dma_start(xgb[:, :nj_x * SB],
                                  xgv_d[:, xoff: xoff + nj_x * SB])

            def emit_trees(b):
                bt0, bt1 = bl[b]
                o0, SB, eab, xgb, aev, exb, scr, scu = blk[b]
                ae_t, ae_o = aev[:].tensor, aev[:].offset
                ae_p = list(aev[:].ap[0])
                M = SBmax  # uniform plane stride in LV / scu
                lv_t, lv_o = LV[:].tensor, LV[:].offset
                lv_p = list(LV[:].ap[0])
                su_t, su_o = scu[:].tensor, scu[:].offset
                su_p = list(scu[:].ap[0])
                lvs = lambda k: LV[:, k * M: k * M + SB]
                sus = lambda k: scu[:, k * M: k * M + SB]

                # U-tree leaves + V-leaves j=5,6 on ACT — overlap DVE leaves
                for h in range(H):
                    for j in range(nj_x):
                        nc.scalar.activation(
                            sus(h * nj_x + j), xgb[:, j * SB:(j + 1) * SB],
                            AF.Copy,
                            scale=scal[:, OFF_US + j * H + h:
                                       OFF_US + j * H + h + 1])
                for h in range(H):
                    for j in (5, 6):
                        nc.scalar.activation(
                            lvs(h * nj_e + j), eab[:, j * SB:(j + 1) * SB],
                            AF.Copy,
                            scale=scal[:, OFF_V + j * H + h:
                                       OFF_V + j * H + h + 1])

                # V-leaves j=0..4 on DVE
                for h in range(H):
                    for j in range(5):
                        nc.vector.tensor_scalar(
                            out=lvs(h * nj_e + j),
                            in0=eab[:, j * SB:(j + 1) * SB],
                            scalar1=scal[:, OFF_V + j * H + h:
                                         OFF_V + j * H + h + 1],
                            scalar2=None, op0=OP.mult)

                # merge tree batched across heads (7 TT instrs per block)
                ap3 = lambda t, o, p, d: AP(t, o, [p] + d)
                # B1: lv[7h+{0,2,4}] += lv[7h+{1,3,5}]
                nc.vector.tensor_tensor(
                    out=ap3(lv_t, lv_o, lv_p,
                            [[nj_e * M, H], [2 * M, 3], [1, SB]]),
                    in0=ap3(lv_t, lv_o, lv_p,
                            [[nj_e * M, H], [2 * M, 3], [1, SB]]),
                    in1=ap3(lv_t, lv_o + M, lv_p,
                            [[nj_e * M, H], [2 * M, 3], [1, SB]]),
                    op=OP.add)
                # B2: lv[7h+4] += lv[7h+6]
                nc.vector.tensor_tensor(
                    out=ap3(lv_t, lv_o + 4 * M, lv_p,
                            [[nj_e * M, H], [1, SB]]),
                    in0=ap3(lv_t, lv_o + 4 * M, lv_p,
                            [[nj_e * M, H], [1, SB]]),
                    in1=ap3(lv_t, lv_o + 6 * M, lv_p,
                            [[nj_e * M, H], [1, SB]]),
                    op=OP.add)
                # B3: lv[7h] += lv[7h+2] ; B4: lv[7h] += lv[7h+4]
                for o1 in (2 * M, 4 * M):
                    nc.vector.tensor_tensor(
                        out=ap3(lv_t, lv_o, lv_p, [[nj_e * M, H], [1, SB]]),
                        in0=ap3(lv_t, lv_o, lv_p, [[nj_e * M, H], [1, SB]]),
                        in1=ap3(lv_t, lv_o + o1, lv_p,
                                [[nj_e * M, H], [1, SB]]),
                        op=OP.add)
                # B5/B6: su[3h] += su[3h+1] ; su[3h] += su[3h+2]
                for o1 in (M, 2 * M):
                    nc.vector.tensor_tensor(
                        out=ap3(su_t, su_o, su_p, [[nj_x * M, H], [1, SB]]),
                        in0=ap3(su_t, su_o, su_p, [[nj_x * M, H], [1, SB]]),
                        in1=ap3(su_t, su_o + o1, su_p,
                                [[nj_x * M, H], [1, SB]]),
                        op=OP.add)
                # B7: ae[h] = lv[7h] + su[3h]
                nc.vector.tensor_tensor(
                    out=ap3(ae_t, ae_o, ae_p, [[SB, H], [1, SB]]),
                    in0=ap3(lv_t, lv_o, lv_p, [[nj_e * M, H], [1, SB]]),
                    in1=ap3(su_t, su_o, su_p, [[nj_x * M, H], [1, SB]]),
                    op=OP.add)

                # self-loop slot0 += mean of incoming V-logits (per chunk)
                for g in chunks_of(bt0, bt1):
                    ta, tb = g * CH, min((g + 1) * CH, T)
                    tcn = tb - ta
                    dt_g = int(D[ta])
                    lt = int(off[ta]) - o0
                    sl0 = AP(ae_t, ae_o + lt,
                             [ae_p, [dt_g, tcn], [SB, H]])
                    avm_b = AP(avm16[:].tensor, avm16[:].offset + ta,
                               [list(avm16[:].ap[0]), [1, tcn], [T, H]])
                    nc.vector.tensor_tensor(out=sl0, in0=sl0, in1=avm_b,
                                            op=OP.add)

            def emit_logits_tail(b):
                """a_dst add (GpSimd) + leaky-relu + exp (ACT)."""
                bt0, bt1 = bl[b]
                o0, SB, eab, xgb, aev, exb, scr, scu = blk[b]
                ae_t, ae_o = aev[:].tensor, aev[:].offset
                ae_p = list(aev[:].ap[0])
                for g in chunks_of(bt0, bt1):
                    ta, tb = g * CH, min((g + 1) * CH, T)
                    tcn = tb - ta
                    dt_g = int(D[ta])
                    lt = int(off[ta]) - o0
                    sl = AP(ae_t, ae_o + lt,
                            [ae_p, [dt_g, tcn], [SB, H], [1, dt_g]])
                    adb = AP(ad16[:].tensor, ad16[:].offset + ta,
                             [list(ad16[:].ap[0]), [1, tcn], [T, H], [0, dt_g]])
                    nc.vector.tensor_tensor(out=sl, in0=sl, in1=adb, op=OP.add)
                nc.scalar.activation(aev[:, :H * SB], aev[:, :H * SB],
                                     AF.Prelu, alpha=NEG_SLOPE)
                nc.scalar.activation(exb[:, :H * SB], aev[:, :H * SB], AF.Exp,
                                     bias=mcol[:, :1])

            def emit_post_chunk(b, g):
                """denominators, weighted messages, aggregation for chunk g."""
                o0, SB, eab, xgb, aev, exb, scr, scu = blk[b]
                ex_t, ex_o = exb[:].tensor, exb[:].offset
                ex_p = list(exb[:].ap[0])
                if True:
                    ta, tb = g * CH, min((g + 1) * CH, T)
                    tcn = tb - ta
                    dt_g = int(D[ta])
                    lt = int(off[ta]) - o0
                    dg = den_g[g]
                    ag = agg_g[g]

                    nc.vector.tensor_reduce(
                        out=AP(dg[:].tensor, dg[:].offset,
                               [list(dg[:].ap[0]), [1, tcn], [CH, H]]),
                        in_=AP(ex_t, ex_o + lt,
                               [ex_p, [dt_g, tcn], [SB, H], [1, dt_g]]),
                        axis=mybir.AxisListType.X, op=OP.add)
                    # den -= npad * exp(prelu(a_dst))   (pad-slot correction)
                    cor_b = AP(corr[:].tensor, corr[:].offset + ta,
                               [list(corr[:].ap[0]), [1, tcn], [T, H]])
                    dg_b = AP(dg[:].tensor, dg[:].offset,
                              [list(dg[:].ap[0]), [1, tcn], [CH, H]])
                    nc.vector.tensor_tensor(out=dg_b, in0=dg_b, in1=cor_b,
                                            op=OP.subtract)

                    msg = mp.tile([P, H * nj_x * CH * Dmax], F16, tag="msg")
                    m_t, m_o, m_p = (msg[:].tensor, msg[:].offset,
                                     list(msg[:].ap[0]))
                    nd = nj_x * dt_g
                    for ti in range(ta, tb):
                        lt_i = int(off[ta]) - o0 + (ti - ta) * dt_g
                        m_ap = AP(m_t, m_o + (ti - ta) * H * nd,
                                  [m_p, [nd, H], [dt_g, nj_x], [1, dt_g]])
                        ealpha = AP(ex_t, ex_o + lt_i,
                                    [ex_p, [SB, H], [0, nj_x], [1, dt_g]])
                        xgs = AP(xgb[:].tensor, xgb[:].offset + lt_i,
                                 [list(xgb[:].ap[0]), [0, H], [SB, nj_x],
                                  [1, dt_g]])
                        nc.vector.tensor_tensor(out=m_ap, in0=ealpha, in1=xgs,
                                                op=OP.mult)
                    nc.vector.tensor_reduce(
                        out=AP(ag[:].tensor, ag[:].offset,
                               [list(ag[:].ap[0]), [1, tcn], [CH, H * nj_x]]),
                        in_=AP(m_t, m_o,
                               [m_p, [H * nd, tcn], [dt_g, H * nj_x],
                                [1, dt_g]]),
                        axis=mybir.AxisListType.X, op=OP.add)

            def emit_phase2_chunk(g, tail=False):
                if True:
                    ta, tb = g * CH, min((g + 1) * CH, T)
                    tcn = tb - ta
                    cw = tcn * P
                    dg = den_g[g]
                    ag = agg_g[g]

                    # alpha-normalize: agg_n = agg / den (fp32)
                    rec = mp.tile([P, H * CH], dt, tag="rec")
                    nc.vector.reciprocal(rec[:], dg[:])
                    agn = mp.tile([P, H * nj_x * CH], dt, tag="agn")
                    ag_b = AP(ag[:].tensor, ag[:].offset,
                              [list(ag[:].ap[0]), [CH * nj_x, H], [CH, nj_x],
                               [1, tcn]])
                    an_b = AP(agn[:].tensor, agn[:].offset,
                              [list(agn[:].ap[0]), [CH * nj_x, H], [CH, nj_x],
                               [1, tcn]])
                    rec_b = AP(rec[:].tensor, rec[:].offset,
                               [list(rec[:].ap[0]), [CH, H], [0, nj_x],
                                [1, tcn]])
                    nc.vector.tensor_tensor(out=an_b, in0=ag_b, in1=rec_b,
                                            op=OP.mult)

                    # transpose agg_n -> [12, cw] then to fp16 for matmul
                    pst = pq.tile([nj_x * H, CW], dt, tag="pst")
                    for ti in range(ta, tb):
                        nc.tensor.transpose(
                            out=pst[:, (ti - ta) * P:(ti - ta + 1) * P],
                            in_=AP(agn[:].tensor, agn[:].offset + (ti - ta),
                                   [list(agn[:].ap[0]), [CH, nj_x * H]]),
                            identity=ident[:])
                    aggT = mp.tile([nj_x * H, CW], F16, tag="aggT")
                    if tail:
                        nc.vector.tensor_copy(aggT[:, :cw], pst[:, :cw])
                    else:
                        nc.scalar.copy(aggT[:, :cw], pst[:, :cw])

                    ps1 = pp.tile([HC, CW], dt, tag="ps1")
                    nc.tensor.matmul(ps1[:, :cw], wpj[:], aggT[:, :cw],
                                     start=True, stop=True)
                    # ELU(z+bg): min(exp(z+bg),1) - 1 + relu(z+bg)
                    r1 = mp.tile([HC, CW], F16, tag="r1")
                    u1 = mp.tile([HC, CW], F16, tag="u1")
                    nc.scalar.activation(r1[:, :cw], ps1[:, :cw], AF.Relu,
                                         bias=bgc)
                    nc.scalar.activation(u1[:, :cw], ps1[:, :cw], AF.Exp,
                                         bias=bgc)
                    nc.vector.tensor_scalar(out=u1[:, :cw], in0=u1[:, :cw],
                                            scalar1=1.0, scalar2=-1.0,
                                            op0=OP.min, op1=OP.add)
                    nc.vector.tensor_tensor(out=r1[:, :cw], in0=r1[:, :cw],
                                            in1=u1[:, :cw], op=OP.add)

                    ps2 = pp.tile([HC, CW], dt, tag="ps2")
                    nc.tensor.matmul(ps2[:, :cw], w1s[:], r1[:, :cw],
                                     start=True, stop=True)
                    h2 = mp.tile([HC, CW], F16, tag="h2")
                    nc.scalar.activation(h2[:, :cw], ps2[:, :cw], AF.Prelu,
                                         bias=b1c, alpha=prelu_alpha)

                    ps3 = pp.tile([lat, CW], dt, tag="ps3")
                    nc.tensor.matmul(ps3[:, :cw], w2s[:], h2[:, :cw],
                                     start=True, stop=True)
                    o3 = mp.tile([lat, CW], dt, tag="o3")
                    if tail:
                        nc.vector.tensor_scalar(
                            out=o3[:, :cw], in0=ps3[:, :cw],
                            scalar1=b2c, scalar2=None, op0=OP.add)
                    else:
                        nc.scalar.activation(o3[:, :cw], ps3[:, :cw],
                                             AF.Identity, bias=b2c)
                    nc.sync.dma_start(out_d[:, ta * P: ta * P + cw],
                                      o3[:, :cw])

            NB = len(bl)
            for b in range(min(2, NB)):
                emit_dma(b)
            # phase-2-only weights: issue after the first input blocks
            nc.sync.dma_start(pk16[:], pack16_d[:])
            nc.sync.dma_start(wpj[:], wpj16_d[:])
            nc.sync.dma_start(ident[:], id_d[:])
            emit_ad_corr()
            emit_trees(0)
            p2q = []  # chunks with post emitted, phase2 pending (lag 2)
            for b in range(NB):
                emit_logits_tail(b)
                if b + 2 < NB:
                    emit_dma(b + 2)
                if b + 1 < NB:
                    emit_trees(b + 1)
                for g in chunks_of(*bl[b]):
                    emit_post_chunk(b, g)
                    p2q.append(g)
                    if len(p2q) > 3:
                        emit_phase2_chunk(p2q.pop(0), tail=(b == NB - 1))
            for g in p2q:
                emit_phase2_chunk(g, tail=True)

    return nc


# ---------------------------------------------------------------------------
# Full kernel entry (host orchestration).
# ---------------------------------------------------------------------------
def make_in_maps(sched, streams, w, n_cores):
    maps = []
    for c in range(n_cores):
        m = dict(
            ea7=streams["ea7"][c].reshape(P, -1),
            xgv=streams["xgv"][c].reshape(P, -1),
            tabs=streams["tabs"][c],
        )
        m.update(w)
        maps.append(m)
    return maps


def unscramble(results, sched, unscr, N, lat=32):
    n_cores = sched["n_cores"]
    T = sched["T"]
    out = np.zeros((N, lat), dtype=np.float32)
    for c in range(n_cores):
        o = results[c]["out"].reshape(lat, T, P).transpose(2, 1, 0)
        node_of = unscr["node_of"][c]  # [T, P] global ids (clamped for dummies)
        valid = unscr["valid_loc"][c].reshape(T, P)
        for t in range(T):
            v = valid[t]
            out[node_of[t][v]] = o[v, t]
    return out


# ---------------------------------------------------------------------------
# Self-contained harness entry: kernel(**inputs) -> full [N, 32] output.
# ---------------------------------------------------------------------------
_CACHE = {}


def kernel(x, edge_index, edge_attr, W_gat, att_src, att_dst, W_edge,
           att_edge, bias_gat, W1, b1, prelu_a, W2, b2):
    from concourse.bass_utils import run_bass_kernel_spmd

    patch_tile_epilogue()
    n_cores = 8
    x = np.asarray(x)
    edge_index = np.asarray(edge_index)
    edge_attr = np.asarray(edge_attr)
    H, C = np.asarray(att_src).shape

    sched, streams, unscr = host_prep(x, edge_index, edge_attr, n_cores)
    w = host_weights(H, C, np.asarray(W_gat), np.asarray(att_src),
                     np.asarray(att_dst), np.asarray(W_edge),
                     np.asarray(att_edge), np.asarray(bias_gat),
                     np.asarray(W1), np.asarray(b1), np.asarray(prelu_a),
                     np.asarray(W2), np.asarray(b2))

    key = (sched["T"], sched["S"], tuple(int(d) for d in sched["D"]),
           float(np.asarray(prelu_a)))
    if key not in _CACHE:
        _CACHE[key] = build_program(sched, n_heads=H,
                                    prelu_alpha=float(np.asarray(prelu_a)))
    nc = _CACHE[key]

    maps = make_in_maps(sched, streams, w, n_cores)
    res = run_bass_kernel_spmd(nc, maps, core_ids=list(range(n_cores)))
    out = unscramble(res.results, sched, unscr, x.shape[0])
    return out.astype(np.float32)


# revision 91
# speedup vs baseline: 2.5022x; 1.0221x over previous
"""GAT encoder Bass kernel for TRN2 — v2.

Architecture: dst-sharded nodes across 8 cores; per-core edge-major
"plane-major" layout [128 node-rows, ch-plane, slot]; degree-sorted 128-node
tiles with shared (max-over-core) slot schedule, slot count UNIFORM within
each 4-tile chunk so per-tile ops batch into one instruction per chunk.
Host ships fp16 halo-expanded source features per slot (x[src]), fp16
edge_attr planes, per-node x, 1/deg, and pad counts.

Device: attention logits via tensor_scalar leaves (4x DVE mode) + fp16
tensor_tensor merge trees; self-loop logit = mean of real edge logits
(per-chunk batched reduce); a_dst broadcast add on GpSimd; leaky-relu+exp on
ACT; per-chunk denominator/aggregation reduces on DVE with a pad-slot
denominator correction (no validity plane); projection 12->128 (block-diag
W_gat fp16), ELU, MLP 128->128 (PReLU) ->32 in ch-major with fp16 PE
matmuls, double-buffered PSUM, per-chunk output DMA.
"""

import numpy as np
import concourse.bass as bass
import concourse.mybir as mybir
import concourse.tile as tile
from concourse.bass import AP

F32 = mybir.dt.float32
F16 = mybir.dt.float16
AF = mybir.ActivationFunctionType
OP = mybir.AluOpType

P = 128
NEG_SLOPE = 0.2
CH = 4  # tiles per chunk (uniform slot count within a chunk)
MSHIFT = -8.0  # logit shift before exp (cancels in softmax; avoids fp16 inf)


# ---------------------------------------------------------------------------
# Tile-framework epilogue fix: this walrus build rejects >=2 sync waits on the
# kernel-tail Drain ("Too many sync wait commands").  Strip the waits off the
# drain and re-emit them as individual sync-engine nops.
# ---------------------------------------------------------------------------
def block_split(T):
    """Chunk-aligned block boundaries shared by host layout and device."""
    NG = (T + CH - 1) // CH
    a = max(1, (NG - 3) * 2 // 5)
    ngs = [2, a, NG - 3 - a, 1]
    bts = [0]
    for n in ngs:
        bts.append(min(bts[-1] + n * CH, T))
    bts[-1] = T
    return [(bts[i], bts[i + 1]) for i in range(len(ngs))
            if bts[i] < bts[i + 1]]


def patch_tile_epilogue():
    from concourse.tile import ScopedClock
    import bass_rust

    if getattr(tile.TileContext, "_gatk_patched", False):
        return

    orig_lower = tile.TileContext._lower_ordered_insts

    def _lower_ordered_insts(self, ordered):
        for bb_name, insts in list(ordered.items()):
            out = []
            for inst in insts:
                si = inst.sync_info
                if si is not None and si.on_wait and len(si.on_wait) > 1:
                    waits = list(si.on_wait)
                    for i, w in enumerate(waits[:-1]):
                        n = bass_rust.InstNoOp(
                            name=f"{inst.name}-sw{i}", ins=[], outs=[])
                        n.engine = inst.engine
                        n.sync_info = mybir.SyncInfo(
                            on_wait=[w], on_update=[])
                        out.append(n)
                    si.on_wait.clear()
                    si.on_wait.append(waits[-1])
                out.append(inst)
            ordered[bb_name] = out
        return orig_lower(self, ordered)

    tile.TileContext._lower_ordered_insts = _lower_ordered_insts
    tile.TileContext._gatk_patched = True

    def _drain_and_barrier(self, tick_clock, wait_clock):
        drain_inst = self.nc.sync.drain()
        wait_clock.add_sem_waits(
            drain_inst.ins, ScopedClock({None: tick_clock.global_clock})
        )
        si = drain_inst.ins.sync_info
        waits = list(si.on_wait or [])
        si.on_wait.clear()
        for w in waits:
            n = self.nc.sync.nop()
            nsi = n.ins.sync_info
            if nsi is None:
                n.ins.sync_info = mybir.SyncInfo(on_wait=[w], on_update=[])
            else:
                nsi.on_wait.append(w)
        self.nc.all_engine_barrier()
        assert self.sems is not None
        popped = self.nc._tile_sem_poison_stack.pop()
        assert popped is self._sem_poison
        self.nc.clear_and_free_semaphores(list(self.sems.allocated().values()))
        self.nc.all_engine_barrier()

    tile.TileContext._drain_and_barrier = _drain_and_barrier


# ---------------------------------------------------------------------------
# Host-side sharding / layout prep (pure indexing + input redistribution).
# ---------------------------------------------------------------------------
def host_prep(x, edge_index, edge_attr, n_cores):
    N = x.shape[0]
    E = edge_index.shape[1]
    NLOC = N // n_cores
    NPAD = ((NLOC + P - 1) // P) * P
    T = NPAD // P

    src = np.asarray(edge_index[0], dtype=np.int64)
    dst = np.asarray(edge_index[1], dtype=np.int64)
    x = np.asarray(x, dtype=np.float32)
    ea = np.asarray(edge_attr, dtype=np.float32)

    deg = np.bincount(dst, minlength=N).astype(np.int64)

    # per-core degree-sorted node order
    orders = np.zeros((n_cores, NPAD), dtype=np.int64)  # sorted-pos -> local id
    ranks = np.zeros((n_cores, NPAD), dtype=np.int64)   # local id -> sorted-pos
    degp = np.zeros((n_cores, NPAD), dtype=np.int64)
    for c in range(n_cores):
        dloc = np.zeros(NPAD, dtype=np.int64)
        dloc[:NLOC] = deg[c * NLOC:(c + 1) * NLOC]
        dloc[NLOC:] = -1  # dummies first
        o = np.argsort(dloc, kind="stable")
        orders[c] = o
        ranks[c, o] = np.arange(NPAD)
        degp[c] = np.maximum(dloc[o], 0)  # sorted-pos -> degree (dummies 0)

    # shared slot schedule; D uniform within each CH-tile chunk
    D = np.zeros(T, dtype=np.int64)
    for t in range(T):
        D[t] = degp[:, t * P:(t + 1) * P].max() + 1
    for g in range((T + CH - 1) // CH):
        t0, t1 = g * CH, min((g + 1) * CH, T)
        D[t0:t1] = D[t0:t1].max()
    off = np.zeros(T + 1, dtype=np.int64)
    off[1:] = np.cumsum(D)
    S = int(off[-1])

    # edge -> (core, p, slot)
    e_core = dst // NLOC
    e_rank = ranks[e_core, dst - e_core * NLOC]
    e_t = e_rank // P
    e_p = e_rank % P
    # within-destination running index (1..deg); self-loop is slot 0
    order_e = np.argsort(dst, kind="stable")
    kk = np.empty(E, dtype=np.int64)
    ds = dst[order_e]
    grp_start = np.r_[0, np.flatnonzero(ds[1:] != ds[:-1]) + 1]
    lengths = np.diff(np.r_[grp_start, E])
    within = np.arange(E) - np.repeat(grp_start, lengths)
    kk[order_e] = within + 1
    e_s = off[e_t] + kk

    ea7 = np.zeros((n_cores, P, 7, S), dtype=np.float32)
    xgv = np.zeros((n_cores, P, 3, S), dtype=np.float32)

    ea7[e_core, e_p, :, e_s] = ea
    xgv[e_core, e_p, :, e_s] = x[src]
    bl = block_split(T)

    # self slots + per-node tables
    xn3 = np.zeros((n_cores, P, 3, T), dtype=np.float32)
    invd = np.zeros((n_cores, P, T), dtype=np.float32)
    npad = np.zeros((n_cores, P, T), dtype=np.float32)
    node_of = np.zeros((n_cores, T, P), dtype=np.int64)
    for c in range(n_cores):
        loc = orders[c]  # sorted-pos -> local id
        glob = c * NLOC + loc
        valid = loc < NLOC
        xg_nodes = np.where(valid[:, None], x[np.minimum(glob, N - 1)], 0.0)
        for t in range(T):
            sl = slice(t * P, (t + 1) * P)
            xn3[c, :, :, t] = xg_nodes[sl]
            xgv[c, :, :, off[t]] = xg_nodes[sl]
            invd[c, :, t] = 1.0 / np.maximum(degp[c, sl], 1)
            npad[c, :, t] = (D[t] - 1) - degp[c, sl]
            node_of[c, t] = glob[sl]

    # block-contiguous shipping layout: per block, planes packed contiguously
    ea7s = np.concatenate(
        [ea7[:, :, :, off[t0]:off[t1]].reshape(n_cores, P, -1)
         for (t0, t1) in bl], axis=2)
    xgvs = np.concatenate(
        [xgv[:, :, :, off[t0]:off[t1]].reshape(n_cores, P, -1)
         for (t0, t1) in bl], axis=2)

    # per-core fp32 tables packed into one tensor: xn3 | invd | npad | easum
    easum = np.add.reduceat(ea7, off[:-1], axis=3)  # [C, P, 7, T]
    tabs = np.concatenate([xn3.reshape(n_cores, P, -1), invd, npad,
                           easum.reshape(n_cores, P, -1)], axis=2)

    sched = dict(T=T, D=D, off=off, S=S, NLOC=NLOC, NPAD=NPAD, n_cores=n_cores)
    streams = dict(ea7=ea7s.astype(np.float16), xgv=xgvs.astype(np.float16),
                   tabs=np.ascontiguousarray(tabs))
    unscr = dict(node_of=node_of, valid_loc=orders < NLOC)
    return sched, streams, unscr


def host_weights(n_heads, C, W_gat, att_src, att_dst, W_edge, att_edge,
                 bias_gat, W1, b1, prelu_a, W2, b2):
    """Pure-layout reshapes/replications/casts of the weight tensors.

    packw [P, 20] fp32:  0:3 W_gat.T | 3:10 W_edge.T | 10 att_src |
      11 att_dst | 12 att_edge | 13:17 hmask | 17 bias_gat | 18 b1 |
      19 b2 (rows 0:32)
    pack16 [P, 160] fp16:  0:128 W1 | 128:160 W2
    """
    HC = n_heads * C
    nj_x = W_gat.shape[0]
    packw = np.zeros((P, 20), dtype=np.float32)
    packw[:, 0:3] = W_gat.T
    packw[:, 3:10] = W_edge.T
    packw[:, 10] = att_src.reshape(HC)
    packw[:, 11] = att_dst.reshape(HC)
    packw[:, 12] = att_edge.reshape(HC)
    for h in range(n_heads):
        packw[h * C:(h + 1) * C, 13 + h] = 1.0
    packw[:, 17] = bias_gat
    packw[:, 18] = b1
    packw[:b2.shape[0], 19] = b2
    pack16 = np.zeros((P, HC + 32), dtype=np.float16)
    pack16[:, 0:HC] = W1
    pack16[:, HC:HC + 32] = W2
    wpj = np.zeros((nj_x * n_heads, HC), dtype=np.float32)
    for h in range(n_heads):
        wpj[nj_x * h: nj_x * (h + 1), C * h: C * (h + 1)] = \
            W_gat[:, C * h: C * (h + 1)]
    return dict(
        packw=packw,
        pack16=pack16,
        wpj16=np.ascontiguousarray(wpj, dtype=np.float16),
        ident=np.eye(P, dtype=np.float32),
    )


# ---------------------------------------------------------------------------
# Device program.
# ---------------------------------------------------------------------------
def build_program(sched, n_heads=4, nj_x=3, nj_e=7, lat=32,
                  prelu_alpha=0.25):
    T = sched["T"]
    D = sched["D"]
    off = sched["off"]
    S = sched["S"]
    HC = P  # hidden dim == 128 == partitions
    H = n_heads
    NG = (T + CH - 1) // CH  # chunk groups

    nc = bass.Bass()
    dt = F32

    # --- dram I/O ---
    ea7_d = nc.dram_tensor("ea7", [P, nj_e * S], F16, kind="ExternalInput")
    xgv_d = nc.dram_tensor("xgv", [P, nj_x * S], F16, kind="ExternalInput")
    tabs_d = nc.dram_tensor("tabs", [P, 12 * T], dt, kind="ExternalInput")
    packw_d = nc.dram_tensor("packw", [P, 20], dt, kind="ExternalInput")
    pack16_d = nc.dram_tensor("pack16", [P, HC + lat], F16,
                              kind="ExternalInput")
    wpj16_d = nc.dram_tensor("wpj16", [nj_x * H, HC], F16, kind="ExternalInput")
    id_d = nc.dram_tensor("ident", [P, P], dt, kind="ExternalInput")
    # output in channel-major [lat, (t, p)]; host transposes in unscramble
    out_d = nc.dram_tensor("out", [lat, T * P], dt, kind="ExternalOutput")

    NSC = nj_e * H + nj_x * H + nj_x * H  # scale columns: V | U_src | U_dst
    OFF_V, OFF_US, OFF_UD = 0, nj_e * H, nj_e * H + nj_x * H

    # blocks split at chunk boundaries; small first block to fill the pipe,
    # small last block to shorten the un-overlapped phase-2 tail
    bl = block_split(T)
    SBmax = max(int(off[t1] - off[t0]) for (t0, t1) in bl)
    Dmax = int(D.max())
    CW = CH * P  # phase-2 chunk column width

    with tile.TileContext(nc) as tc:
        with (
            tc.tile_pool(name="wp", bufs=1) as wp,
            tc.tile_pool(name="sp", bufs=2) as sp,
            tc.tile_pool(name="mp", bufs=3) as mp,
            tc.tile_pool(name="pp", bufs=2, space="PSUM") as pp,
            tc.tile_pool(name="pq", bufs=1, space="PSUM") as pq,
        ):
            # ---------------- phase 0: weights & derived ----------------
            pw = wp.tile([P, 20], dt, tag="pw")
            tabs = wp.tile([P, 12 * T], dt, tag="tabs")
            pk16 = wp.tile([P, HC + lat], F16, tag="pk16")
            wpj = wp.tile([nj_x * H, HC], F16, tag="wpj")
            ident = wp.tile([P, P], dt, tag="ident")
            nc.sync.dma_start(pw[:], packw_d[:])
            nc.sync.dma_start(tabs[:], tabs_d[:])
            # slices of the packed tensors
            wgT = pw[:, 0:3]
            weT = pw[:, 3:10]
            asc = pw[:, 10:11]
            adc = pw[:, 11:12]
            aec = pw[:, 12:13]
            bgc = pw[:, 17:18]
            b1c = pw[:, 18:19]
            b2c = pw[0:lat, 19:20]
            w1s = pk16[:, 0:HC]
            w2s = pk16[:, HC:HC + lat]
            XNS0, IVD0, NPD0, EAS0 = 0, 3 * T, 4 * T, 5 * T
            onesr = wp.tile([1, P], dt, tag="onesr")
            nc.vector.memset(onesr[:], 1.0)
            mcol = wp.tile([P, 1], dt, tag="mcol")
            nc.vector.memset(mcol[:], MSHIFT)

            # W28 = W_edgeT (j-major x H) * head-mask ; W12 same from W_gatT
            w28 = wp.tile([HC, nj_e * H], dt, tag="w28")
            w12 = wp.tile([HC, nj_x * H], dt, tag="w12")
            pw_t, pw_o = pw[:].tensor, pw[:].offset
            pw_p = list(pw[:].ap[0])
            weT_b = AP(pw_t, pw_o + 3, [pw_p, [1, nj_e], [0, H]])
            hm_e = AP(pw_t, pw_o + 13, [pw_p, [0, nj_e], [1, H]])
            nc.vector.tensor_tensor(
                out=w28[:].rearrange("p (j h) -> p j h", j=nj_e),
                in0=weT_b, in1=hm_e, op=OP.mult)
            wgT_b = AP(pw_t, pw_o + 0, [pw_p, [1, nj_x], [0, H]])
            hm_x = AP(pw_t, pw_o + 13, [pw_p, [0, nj_x], [1, H]])
            nc.vector.tensor_tensor(
                out=w12[:].rearrange("p (j h) -> p j h", j=nj_x),
                in0=wgT_b, in1=hm_x, op=OP.mult)

            # scale rows via K=128 matmuls, then partition-broadcast
            ps1w = pp.tile([HC, CW], dt, tag="ps1")
            ps2w = pp.tile([HC, CW], dt, tag="ps2")
            srow = wp.tile([1, NSC], dt, tag="srow")
            psv = ps1w[0:1, 0:NSC]
            nc.tensor.matmul(psv[:, 0:nj_e * H], aec, w28[:],
                             start=True, stop=True)
            nc.tensor.matmul(psv[:, OFF_US:OFF_US + nj_x * H], asc, w12[:],
                             start=True, stop=True)
            nc.tensor.matmul(psv[:, OFF_UD:OFF_UD + nj_x * H], adc, w12[:],
                             start=True, stop=True)
            nc.vector.tensor_copy(srow[:], psv)
            scal = wp.tile([P, NSC], dt, tag="scal")
            psb = ps2w[0:P, 0:NSC]
            nc.tensor.matmul(psb, onesr[:], srow[:], start=True, stop=True)
            nc.vector.tensor_copy(scal[:], psb)

            ad_all = wp.tile([P, H * T], dt, tag="ad_all")
            ad16 = wp.tile([P, H * T], F16, tag="ad16")
            corr = wp.tile([P, H * T], dt, tag="corr")
            tpr = wp.tile([P, H * T], F16, tag="tpr")
            av_all = wp.tile([P, H * T], dt, tag="av_all")
            avm16 = wp.tile([P, H * T], F16, tag="avm16")
            LV = wp.tile([P, H * nj_e * SBmax], F16, tag="LV")

            def emit_ad_corr():
                # ad_all [P, H, T] from xn planes (a_dst per node)
                for h in range(H):
                    adh = ad_all[:, h * T:(h + 1) * T]
                    nc.vector.tensor_scalar(
                        out=adh, in0=tabs[:, XNS0:XNS0 + T],
                        scalar1=scal[:, OFF_UD + h: OFF_UD + h + 1],
                        scalar2=None, op0=OP.mult)
                    for j in range(1, nj_x):
                        nc.vector.scalar_tensor_tensor(
                            out=adh,
                            in0=tabs[:, XNS0 + j * T: XNS0 + (j + 1) * T],
                            scalar=scal[:, OFF_UD + j * H + h:
                                        OFF_UD + j * H + h + 1],
                            in1=adh, op0=OP.mult, op1=OP.add)
                nc.vector.tensor_copy(ad16[:], ad_all[:])
                # pad-slot denominator correction: corr = npad*exp(prelu(ad))
                nc.scalar.activation(tpr[:], ad16[:], AF.Prelu,
                                     alpha=NEG_SLOPE)
                nc.scalar.activation(corr[:], tpr[:], AF.Exp,
                                     bias=mcol[:, :1])
                npd_b = AP(tabs[:].tensor, tabs[:].offset + NPD0,
                           [list(tabs[:].ap[0]), [0, H], [1, T]])
                nc.vector.tensor_tensor(out=corr[:].rearrange(
                    "p (h t) -> p h t", h=H), in0=corr[:].rearrange(
                    "p (h t) -> p h t", h=H), in1=npd_b, op=OP.mult)
                # self-loop logit base: mean of incoming V-logits per node
                # av_all[p,h,t] = sum_j easum_j * V[j,h];  avm16 = av * invd
                for h in range(H):
                    avh = av_all[:, h * T:(h + 1) * T]
                    nc.vector.tensor_scalar(
                        out=avh, in0=tabs[:, EAS0:EAS0 + T],
                        scalar1=scal[:, OFF_V + h: OFF_V + h + 1],
                        scalar2=None, op0=OP.mult)
                    for j in range(1, nj_e):
                        nc.vector.scalar_tensor_tensor(
                            out=avh,
                            in0=tabs[:, EAS0 + j * T: EAS0 + (j + 1) * T],
                            scalar=scal[:, OFF_V + j * H + h:
                                        OFF_V + j * H + h + 1],
                            in1=avh, op0=OP.mult, op1=OP.add)
                ivd_f = AP(tabs[:].tensor, tabs[:].offset + IVD0,
                           [list(tabs[:].ap[0]), [0, H], [1, T]])
                nc.vector.tensor_tensor(out=avm16[:].rearrange(
                    "p (h t) -> p h t", h=H), in0=av_all[:].rearrange(
                    "p (h t) -> p h t", h=H), in1=ivd_f, op=OP.mult)

            # persistent per-chunk accumulators
            den_g = [wp.tile([P, H * CH], dt, tag=f"den{g}", name=f"den{g}")
                     for g in range(NG)]
            agg_g = [wp.tile([P, H * nj_x * CH], dt, tag=f"agg{g}",
                             name=f"agg{g}") for g in range(NG)]

            def chunks_of(bt0, bt1):
                return range(bt0 // CH, (bt1 + CH - 1) // CH)

            # ------------- per-block edge pipeline (sw-pipelined) ----------
            # Emission order: trees(0) | [logits-tail(b) ; trees(b+1) ;
            # post-exp(b) ; phase2(b)] so DVE never waits on pool/ACT.
            blk = {}

            def emit_dma(b):
                bt0, bt1 = bl[b]
                o0, o1 = int(off[bt0]), int(off[bt1])
                SB = o1 - o0
                # block-contiguous dram offsets
                eoff = sum(nj_e * (int(off[t1]) - int(off[t0]))
                           for (t0, t1) in bl[:b])
                xoff = sum(nj_x * (int(off[t1]) - int(off[t0]))
                           for (t0, t1) in bl[:b])
                eab = sp.tile([P, nj_e * SBmax], F16, tag="eab")
                xgb = sp.tile([P, nj_x * SBmax], F16, tag="xgb")
                aev = sp.tile([P, H * SBmax], F16, tag="aev")
                exb = sp.tile([P, H * SBmax], F16, tag="exb")
                scu = sp.tile([P, H * nj_x * SBmax], F16, tag="scu")
                blk[b] = (o0, SB, eab, xgb, aev, exb, None, scu)
                half = (nj_e * SB) // 2
                nc.sync.dma_start(eab[:, :half], ea7_d[:, eoff: eoff + half])
                nc.sync.dma_start(eab[:, half: nj_e * SB],
                                  ea7_d[:, eoff + half: eoff + nj_e * SB])
                nc.sync.dma_start(xgb[:, :nj_x * SB],
                                  xgv_d[:, xoff: xoff + nj_x * SB])

            def emit_trees(b):
                bt0, bt1 = bl[b]
                o0, SB, eab, xgb, aev, exb, scr, scu = blk[b]
                ae_t, ae_o = aev[:].tensor, aev[:].offset
                ae_p = list(aev[:].ap[0])
                M = SBmax  # uniform plane stride in LV / scu
                lv_t, lv_o = LV[:].tensor, LV[:].offset
                lv_p = list(LV[:].ap[0])
                su_t, su_o = scu[:].tensor, scu[:].offset
                su_p = list(scu[:].ap[0])
                lvs = lambda k: LV[:, k * M: k * M + SB]
                sus = lambda k: scu[:, k * M: k * M + SB]

                # U-tree leaves + V-leaves j=5,6 on ACT — overlap DVE leaves
                for h in range(H):
                    for j in range(nj_x):
                        nc.scalar.activation(
                            sus(h * nj_x + j), xgb[:, j * SB:(j + 1) * SB],
                            AF.Copy,
                            scale=scal[:, OFF_US + j * H + h:
                                       OFF_US + j * H + h + 1])
                for h in range(H):
                    for j in (5, 6):
                        nc.scalar.activation(
                            lvs(h * nj_e + j), eab[:, j * SB:(j + 1) * SB],
                            AF.Copy,
                            scale=scal[:, OFF_V + j * H + h:
                                       OFF_V + j * H + h + 1])

                # V-leaves j=0..4 on DVE
                for h in range(H):
                    for j in range(5):
                        nc.vector.tensor_scalar(
                            out=lvs(h * nj_e + j),
                            in0=eab[:, j * SB:(j + 1) * SB],
                            scalar1=scal[:, OFF_V + j * H + h:
                                         OFF_V + j * H + h + 1],
                            scalar2=None, op0=OP.mult)

                # merge tree batched across heads (7 TT instrs per block)
                ap3 = lambda t, o, p, d: AP(t, o, [p] + d)
                # B1: lv[7h+{0,2,4}] += lv[7h+{1,3,5}]
                nc.vector.tensor_tensor(
                    out=ap3(lv_t, lv_o, lv_p,
                            [[nj_e * M, H], [2 * M, 3], [1, SB]]),
                    in0=ap3(lv_t, lv_o, lv_p,
                            [[nj_e * M, H], [2 * M, 3], [1, SB]]),
                    in1=ap3(lv_t, lv_o + M, lv_p,
                            [[nj_e * M, H], [2 * M, 3], [1, SB]]),
                    op=OP.add)
                # B2: lv[7h+4] += lv[7h+6]
                nc.vector.tensor_tensor(
                    out=ap3(lv_t, lv_o + 4 * M, lv_p,
                            [[nj_e * M, H], [1, SB]]),
                    in0=ap3(lv_t, lv_o + 4 * M, lv_p,
                            [[nj_e * M, H], [1, SB]]),
                    in1=ap3(lv_t, lv_o + 6 * M, lv_p,
                            [[nj_e * M, H], [1, SB]]),
                    op=OP.add)
                # B3: lv[7h] += lv[7h+2] ; B4: lv[7h] += lv[7h+4]
                for o1 in (2 * M, 4 * M):
                    nc.vector.tensor_tensor(
                        out=ap3(lv_t, lv_o, lv_p, [[nj_e * M, H], [1, SB]]),
                        in0=ap3(lv_t, lv_o, lv_p, [[nj_e * M, H], [1, SB]]),
                        in1=ap3(lv_t, lv_o + o1, lv_p,
                                [[nj_e * M, H], [1, SB]]),
                        op=OP.add)
                # B5/B6: su[3h] += su[3h+1] ; su[3h] += su[3h+2]
                for o1 in (M, 2 * M):
                    nc.vector.tensor_tensor(
                        out=ap3(su_t, su_o, su_p, [[nj_x * M, H], [1, SB]]),
                        in0=ap3(su_t, su_o, su_p, [[nj_x * M, H], [1, SB]]),
                        in1=ap3(su_t, su_o + o1, su_p,
                                [[nj_x * M, H], [1, SB]]),
                        op=OP.add)
                # B7: ae[h] = lv[7h] + su[3h]
                nc.vector.tensor_tensor(
                    out=ap3(ae_t, ae_o, ae_p, [[SB, H], [1, SB]]),
                    in0=ap3(lv_t, lv_o, lv_p, [[nj_e * M, H], [1, SB]]),
                    in1=ap3(su_t, su_o, su_p, [[nj_x * M, H], [1, SB]]),
                    op=OP.add)

                # self-loop slot0 += mean of incoming V-logits (per chunk)
                for g in chunks_of(bt0, bt1):
                    ta, tb = g * CH, min((g + 1) * CH, T)
                    tcn = tb - ta
                    dt_g = int(D[ta])
                    lt = int(off[ta]) - o0
                    sl0 = AP(ae_t, ae_o + lt,
                             [ae_p, [dt_g, tcn], [SB, H]])
                    avm_b = AP(avm16[:].tensor, avm16[:].offset + ta,
                               [list(avm16[:].ap[0]), [1, tcn], [T, H]])
                    nc.vector.tensor_tensor(out=sl0, in0=sl0, in1=avm_b,
                                            op=OP.add)

            def emit_logits_tail(b):
                """a_dst add (GpSimd) + leaky-relu + exp (ACT)."""
                bt0, bt1 = bl[b]
                o0, SB, eab, xgb, aev, exb, scr, scu = blk[b]
                ae_t, ae_o = aev[:].tensor, aev[:].offset
                ae_p = list(aev[:].ap[0])
                for g in chunks_of(bt0, bt1):
                    ta, tb = g * CH, min((g + 1) * CH, T)
                    tcn = tb - ta
                    dt_g = int(D[ta])
                    lt = int(off[ta]) - o0
                    sl = AP(ae_t, ae_o + lt,
                            [ae_p, [dt_g, tcn], [SB, H], [1, dt_g]])
                    adb = AP(ad16[:].tensor, ad16[:].offset + ta,
                             [list(ad16[:].ap[0]), [1, tcn], [T, H], [0, dt_g]])
                    nc.vector.tensor_tensor(out=sl, in0=sl, in1=adb, op=OP.add)
                nc.scalar.activation(aev[:, :H * SB], aev[:, :H * SB],
                                     AF.Prelu, alpha=NEG_SLOPE)
                nc.scalar.activation(exb[:, :H * SB], aev[:, :H * SB], AF.Exp,
                                     bias=mcol[:, :1])

            def emit_post_chunk(b, g):
                """denominators, weighted messages, aggregation for chunk g."""
                o0, SB, eab, xgb, aev, exb, scr, scu = blk[b]
                ex_t, ex_o = exb[:].tensor, exb[:].offset
                ex_p = list(exb[:].ap[0])
                if True:
                    ta, tb = g * CH, min((g + 1) * CH, T)
                    tcn = tb - ta
                    dt_g = int(D[ta])
                    lt = int(off[ta]) - o0
                    dg = den_g[g]
                    ag = agg_g[g]

                    nc.vector.tensor_reduce(
                        out=AP(dg[:].tensor, dg[:].offset,
                               [list(dg[:].ap[0]), [1, tcn], [CH, H]]),
                        in_=AP(ex_t, ex_o + lt,
                               [ex_p, [dt_g, tcn], [SB, H], [1, dt_g]]),
                        axis=mybir.AxisListType.X, op=OP.add)
                    # den -= npad * exp(prelu(a_dst))   (pad-slot correction)
                    cor_b = AP(corr[:].tensor, corr[:].offset + ta,
                               [list(corr[:].ap[0]), [1, tcn], [T, H]])
                    dg_b = AP(dg[:].tensor, dg[:].offset,
                              [list(dg[:].ap[0]), [1, tcn], [CH, H]])
                    nc.vector.tensor_tensor(out=dg_b, in0=dg_b, in1=cor_b,
                                            op=OP.subtract)

                    msg = mp.tile([P, H * nj_x * CH * Dmax], F16, tag="msg")
                    m_t, m_o, m_p = (msg[:].tensor, msg[:].offset,
                                     list(msg[:].ap[0]))
                    nd = nj_x * dt_g
                    for ti in range(ta, tb):
                        lt_i = int(off[ta]) - o0 + (ti - ta) * dt_g
                        m_ap = AP(m_t, m_o + (ti - ta) * H * nd,
                                  [m_p, [nd, H], [dt_g, nj_x], [1, dt_g]])
                        ealpha = AP(ex_t, ex_o + lt_i,
                                    [ex_p, [SB, H], [0, nj_x], [1, dt_g]])
                        xgs = AP(xgb[:].tensor, xgb[:].offset + lt_i,
                                 [list(xgb[:].ap[0]), [0, H], [SB, nj_x],
                                  [1, dt_g]])
                        nc.vector.tensor_tensor(out=m_ap, in0=ealpha, in1=xgs,
                                                op=OP.mult)
                    nc.vector.tensor_reduce(
                        out=AP(ag[:].tensor, ag[:].offset,
                               [list(ag[:].ap[0]), [1, tcn], [CH, H * nj_x]]),
                        in_=AP(m_t, m_o,
                               [m_p, [H * nd, tcn], [dt_g, H * nj_x],
                                [1, dt_g]]),
                        axis=mybir.AxisListType.X, op=OP.add)

            def emit_phase2_chunk(g, tail=False):
                if True:
                    ta, tb = g * CH, min((g + 1) * CH, T)
                    tcn = tb - ta
                    cw = tcn * P
                    dg = den_g[g]
                    ag = agg_g[g]

                    # alpha-normalize: agg_n = agg / den (fp32)
                    rec = mp.tile([P, H * CH], dt, tag="rec")
                    nc.vector.reciprocal(rec[:], dg[:])
                    agn = mp.tile([P, H * nj_x * CH], dt, tag="agn")
                    ag_b = AP(ag[:].tensor, ag[:].offset,
                              [list(ag[:].ap[0]), [CH * nj_x, H], [CH, nj_x],
                               [1, tcn]])
                    an_b = AP(agn[:].tensor, agn[:].offset,
                              [list(agn[:].ap[0]), [CH * nj_x, H], [CH, nj_x],
                               [1, tcn]])
                    rec_b = AP(rec[:].tensor, rec[:].offset,
                               [list(rec[:].ap[0]), [CH, H], [0, nj_x],
                                [1, tcn]])
                    nc.vector.tensor_tensor(out=an_b, in0=ag_b, in1=rec_b,
                                            op=OP.mult)

                    # transpose agg_n -> [12, cw] then to fp16 for matmul
                    pst = pq.tile([nj_x * H, CW], dt, tag="pst")
                    for ti in range(ta, tb):
                        nc.tensor.transpose(
                            out=pst[:, (ti - ta) * P:(ti - ta + 1) * P],
                            in_=AP(agn[:].tensor, agn[:].offset + (ti - ta),
                                   [list(agn[:].ap[0]), [CH, nj_x * H]]),
                            identity=ident[:])
                    aggT = mp.tile([nj_x * H, CW], F16, tag="aggT")
                    if tail:
                        nc.vector.tensor_copy(aggT[:, :cw], pst[:, :cw])
                    else:
                        nc.scalar.copy(aggT[:, :cw], pst[:, :cw])

                    ps1 = pp.tile([HC, CW], dt, tag="ps1")
                    nc.tensor.matmul(ps1[:, :cw], wpj[:], aggT[:, :cw],
                                     start=True, stop=True)
                    # ELU(z+bg): min(exp(z+bg),1) - 1 + relu(z+bg)
                    r1 = mp.tile([HC, CW], F16, tag="r1")
                    u1 = mp.tile([HC, CW], F16, tag="u1")
                    nc.scalar.activation(r1[:, :cw], ps1[:, :cw], AF.Relu,
                                         bias=bgc)
                    nc.scalar.activation(u1[:, :cw], ps1[:, :cw], AF.Exp,
                                         bias=bgc)
                    nc.vector.tensor_scalar(out=u1[:, :cw], in0=u1[:, :cw],
                                            scalar1=1.0, scalar2=-1.0,
                                            op0=OP.min, op1=OP.add)
                    nc.vector.tensor_tensor(out=r1[:, :cw], in0=r1[:, :cw],
                                            in1=u1[:, :cw], op=OP.add)

                    ps2 = pp.tile([HC, CW], dt, tag="ps2")
                    nc.tensor.matmul(ps2[:, :cw], w1s[:], r1[:, :cw],
                                     start=True, stop=True)
                    h2 = mp.tile([HC, CW], F16, tag="h2")
                    nc.scalar.activation(h2[:, :cw], ps2[:, :cw], AF.Prelu,
                                         bias=b1c, alpha=prelu_alpha)

                    ps3 = pp.tile([lat, CW], dt, tag="ps3")
                    nc.tensor.matmul(ps3[:, :cw], w2s[:], h2[:, :cw],
                                     start=True, stop=True)
                    o3 = mp.tile([lat, CW], dt, tag="o3")
                    if tail:
                        nc.vector.tensor_scalar(
                            out=o3[:, :cw], in0=ps3[:, :cw],
                            scalar1=b2c, scalar2=None, op0=OP.add)
                    else:
                        nc.scalar.activation(o3[:, :cw], ps3[:, :cw],
                                             AF.Identity, bias=b2c)
                    nc.sync.dma_start(out_d[:, ta * P: ta * P + cw],
                                      o3[:, :cw])

            NB = len(bl)
            for b in range(min(2, NB)):
                emit_dma(b)
            # phase-2-only weights: issue after the first input blocks
            nc.sync.dma_start(pk16[:], pack16_d[:])
            nc.sync.dma_start(wpj[:], wpj16_d[:])
            nc.sync.dma_start(ident[:], id_d[:])
            emit_ad_corr()
            emit_trees(0)
            p2q = []  # chunks with post emitted, phase2 pending (lag 2)
            for b in range(NB):
                emit_logits_tail(b)
                if b + 2 < NB:
                    emit_dma(b + 2)
                if b + 1 < NB:
                    emit_trees(b + 1)
                for g in chunks_of(*bl[b]):
                    emit_post_chunk(b, g)
                    p2q.append(g)
                    if len(p2q) > 3:
                        emit_phase2_chunk(p2q.pop(0), tail=(b == NB - 1))
            for g in p2q:
                emit_phase2_chunk(g, tail=True)

    return nc


# ---------------------------------------------------------------------------
# Full kernel entry (host orchestration).
# ---------------------------------------------------------------------------
def make_in_maps(sched, streams, w, n_cores):
    maps = []
    for c in range(n_cores):
        m = dict(
            ea7=streams["ea7"][c].reshape(P, -1),
            xgv=streams["xgv"][c].reshape(P, -1),
            tabs=streams["tabs"][c],
        )
        m.update(w)
        maps.append(m)
    return maps


def unscramble(results, sched, unscr, N, lat=32):
    n_cores = sched["n_cores"]
    T = sched["T"]
    out = np.zeros((N, lat), dtype=np.float32)
    for c in range(n_cores):
        o = results[c]["out"].reshape(lat, T, P).transpose(2, 1, 0)
        node_of = unscr["node_of"][c]  # [T, P] global ids (clamped for dummies)
        valid = unscr["valid_loc"][c].reshape(T, P)
        for t in range(T):
            v = valid[t]
            out[node_of[t][v]] = o[v, t]
    return out


# ---------------------------------------------------------------------------
# Self-contained harness entry: kernel(**inputs) -> full [N, 32] output.
# ---------------------------------------------------------------------------
_CACHE = {}


def kernel(x, edge_index, edge_attr, W_gat, att_src, att_dst, W_edge,
           att_edge, bias_gat, W1, b1, prelu_a, W2, b2):
    from concourse.bass_utils import run_bass_kernel_spmd

    patch_tile_epilogue()
    n_cores = 8
    x = np.asarray(x)
    edge_index = np.asarray(edge_index)
    edge_attr = np.asarray(edge_attr)
    H, C = np.asarray(att_src).shape

    sched, streams, unscr = host_prep(x, edge_index, edge_attr, n_cores)
    w = host_weights(H, C, np.asarray(W_gat), np.asarray(att_src),
                     np.asarray(att_dst), np.asarray(W_edge),
                     np.asarray(att_edge), np.asarray(bias_gat),
                     np.asarray(W1), np.asarray(b1), np.asarray(prelu_a),
                     np.asarray(W2), np.asarray(b2))

    key = (sched["T"], sched["S"], tuple(int(d) for d in sched["D"]),
           float(np.asarray(prelu_a)))
    if key not in _CACHE:
        _CACHE[key] = build_program(sched, n_heads=H,
                                    prelu_alpha=float(np.asarray(prelu_a)))
    nc = _CACHE[key]

    maps = make_in_maps(sched, streams, w, n_cores)
    res = run_bass_kernel_spmd(nc, maps, core_ids=list(range(n_cores)))
    out = unscramble(res.results, sched, unscr, x.shape[0])
    return out.astype(np.float32)


# revision 95
# speedup vs baseline: 2.5253x; 1.0092x over previous
"""GAT encoder Bass kernel for TRN2 — v2.

Architecture: dst-sharded nodes across 8 cores; per-core edge-major
"plane-major" layout [128 node-rows, ch-plane, slot]; degree-sorted 128-node
tiles with shared (max-over-core) slot schedule, slot count UNIFORM within
each 4-tile chunk so per-tile ops batch into one instruction per chunk.
Host ships fp16 halo-expanded source features per slot (x[src]), fp16
edge_attr planes, per-node x, 1/deg, and pad counts.

Device: attention logits via tensor_scalar leaves (4x DVE mode) + fp16
tensor_tensor merge trees; self-loop logit = mean of real edge logits
(per-chunk batched reduce); a_dst broadcast add on GpSimd; leaky-relu+exp on
ACT; per-chunk denominator/aggregation reduces on DVE with a pad-slot
denominator correction (no validity plane); projection 12->128 (block-diag
W_gat fp16), ELU, MLP 128->128 (PReLU) ->32 in ch-major with fp16 PE
matmuls, double-buffered PSUM, per-chunk output DMA.
"""

import numpy as np
import concourse.bass as bass
import concourse.mybir as mybir
import concourse.tile as tile
from concourse.bass import AP

F32 = mybir.dt.float32
F16 = mybir.dt.float16
AF = mybir.ActivationFunctionType
OP = mybir.AluOpType

P = 128
NEG_SLOPE = 0.2
CH = 4  # tiles per chunk (uniform slot count within a chunk)
MSHIFT = -8.0  # logit shift before exp (cancels in softmax; avoids fp16 inf)


# ---------------------------------------------------------------------------
# Tile-framework epilogue fix: this walrus build rejects >=2 sync waits on the
# kernel-tail Drain ("Too many sync wait commands").  Strip the waits off the
# drain and re-emit them as individual sync-engine nops.
# ---------------------------------------------------------------------------
def block_split(T):
    """Chunk-aligned block boundaries shared by host layout and device."""
    NG = (T + CH - 1) // CH
    a = max(1, (NG - 3) // 2)
    ngs = [2, a, NG - 3 - a, 1]
    bts = [0]
    for n in ngs:
        bts.append(min(bts[-1] + n * CH, T))
    bts[-1] = T
    return [(bts[i], bts[i + 1]) for i in range(len(ngs))
            if bts[i] < bts[i + 1]]


def patch_tile_epilogue():
    from concourse.tile import ScopedClock
    import bass_rust

    if getattr(tile.TileContext, "_gatk_patched", False):
        return

    orig_lower = tile.TileContext._lower_ordered_insts

    def _lower_ordered_insts(self, ordered):
        for bb_name, insts in list(ordered.items()):
            out = []
            for inst in insts:
                si = inst.sync_info
                if si is not None and si.on_wait and len(si.on_wait) > 1:
                    waits = list(si.on_wait)
                    for i, w in enumerate(waits[:-1]):
                        n = bass_rust.InstNoOp(
                            name=f"{inst.name}-sw{i}", ins=[], outs=[])
                        n.engine = inst.engine
                        n.sync_info = mybir.SyncInfo(
                            on_wait=[w], on_update=[])
                        out.append(n)
                    si.on_wait.clear()
                    si.on_wait.append(waits[-1])
                out.append(inst)
            ordered[bb_name] = out
        return orig_lower(self, ordered)

    tile.TileContext._lower_ordered_insts = _lower_ordered_insts
    tile.TileContext._gatk_patched = True

    def _drain_and_barrier(self, tick_clock, wait_clock):
        drain_inst = self.nc.sync.drain()
        wait_clock.add_sem_waits(
            drain_inst.ins, ScopedClock({None: tick_clock.global_clock})
        )
        si = drain_inst.ins.sync_info
        waits = list(si.on_wait or [])
        si.on_wait.clear()
        for w in waits:
            n = self.nc.sync.nop()
            nsi = n.ins.sync_info
            if nsi is None:
                n.ins.sync_info = mybir.SyncInfo(on_wait=[w], on_update=[])
            else:
                nsi.on_wait.append(w)
        self.nc.all_engine_barrier()
        assert self.sems is not None
        popped = self.nc._tile_sem_poison_stack.pop()
        assert popped is self._sem_poison
        self.nc.clear_and_free_semaphores(list(self.sems.allocated().values()))
        self.nc.all_engine_barrier()

    tile.TileContext._drain_and_barrier = _drain_and_barrier


# ---------------------------------------------------------------------------
# Host-side sharding / layout prep (pure indexing + input redistribution).
# ---------------------------------------------------------------------------
def host_prep(x, edge_index, edge_attr, n_cores):
    N = x.shape[0]
    E = edge_index.shape[1]
    NLOC = N // n_cores
    NPAD = ((NLOC + P - 1) // P) * P
    T = NPAD // P

    src = np.asarray(edge_index[0], dtype=np.int64)
    dst = np.asarray(edge_index[1], dtype=np.int64)
    x = np.asarray(x, dtype=np.float32)
    ea = np.asarray(edge_attr, dtype=np.float32)

    deg = np.bincount(dst, minlength=N).astype(np.int64)

    # per-core degree-sorted node order
    orders = np.zeros((n_cores, NPAD), dtype=np.int64)  # sorted-pos -> local id
    ranks = np.zeros((n_cores, NPAD), dtype=np.int64)   # local id -> sorted-pos
    degp = np.zeros((n_cores, NPAD), dtype=np.int64)
    for c in range(n_cores):
        dloc = np.zeros(NPAD, dtype=np.int64)
        dloc[:NLOC] = deg[c * NLOC:(c + 1) * NLOC]
        dloc[NLOC:] = -1  # dummies first
        o = np.argsort(dloc, kind="stable")
        orders[c] = o
        ranks[c, o] = np.arange(NPAD)
        degp[c] = np.maximum(dloc[o], 0)  # sorted-pos -> degree (dummies 0)

    # shared slot schedule; D uniform within each CH-tile chunk
    D = np.zeros(T, dtype=np.int64)
    for t in range(T):
        D[t] = degp[:, t * P:(t + 1) * P].max() + 1
    for g in range((T + CH - 1) // CH):
        t0, t1 = g * CH, min((g + 1) * CH, T)
        D[t0:t1] = D[t0:t1].max()
    off = np.zeros(T + 1, dtype=np.int64)
    off[1:] = np.cumsum(D)
    S = int(off[-1])

    # edge -> (core, p, slot)
    e_core = dst // NLOC
    e_rank = ranks[e_core, dst - e_core * NLOC]
    e_t = e_rank // P
    e_p = e_rank % P
    # within-destination running index (1..deg); self-loop is slot 0
    order_e = np.argsort(dst, kind="stable")
    kk = np.empty(E, dtype=np.int64)
    ds = dst[order_e]
    grp_start = np.r_[0, np.flatnonzero(ds[1:] != ds[:-1]) + 1]
    lengths = np.diff(np.r_[grp_start, E])
    within = np.arange(E) - np.repeat(grp_start, lengths)
    kk[order_e] = within + 1
    e_s = off[e_t] + kk

    ea7 = np.zeros((n_cores, P, 7, S), dtype=np.float32)
    xgv = np.zeros((n_cores, P, 3, S), dtype=np.float32)

    ea7[e_core, e_p, :, e_s] = ea
    xgv[e_core, e_p, :, e_s] = x[src]
    bl = block_split(T)

    # self slots + per-node tables
    xn3 = np.zeros((n_cores, P, 3, T), dtype=np.float32)
    invd = np.zeros((n_cores, P, T), dtype=np.float32)
    npad = np.zeros((n_cores, P, T), dtype=np.float32)
    node_of = np.zeros((n_cores, T, P), dtype=np.int64)
    for c in range(n_cores):
        loc = orders[c]  # sorted-pos -> local id
        glob = c * NLOC + loc
        valid = loc < NLOC
        xg_nodes = np.where(valid[:, None], x[np.minimum(glob, N - 1)], 0.0)
        for t in range(T):
            sl = slice(t * P, (t + 1) * P)
            xn3[c, :, :, t] = xg_nodes[sl]
            xgv[c, :, :, off[t]] = xg_nodes[sl]
            invd[c, :, t] = 1.0 / np.maximum(degp[c, sl], 1)
            npad[c, :, t] = (D[t] - 1) - degp[c, sl]
            node_of[c, t] = glob[sl]

    # block-contiguous shipping layout: per block, planes packed contiguously
    ea7s = np.concatenate(
        [ea7[:, :, :, off[t0]:off[t1]].reshape(n_cores, P, -1)
         for (t0, t1) in bl], axis=2)
    xgvs = np.concatenate(
        [xgv[:, :, :, off[t0]:off[t1]].reshape(n_cores, P, -1)
         for (t0, t1) in bl], axis=2)

    # per-core fp32 tables packed into one tensor: xn3 | invd | npad | easum
    easum = np.add.reduceat(ea7, off[:-1], axis=3)  # [C, P, 7, T]
    tabs = np.concatenate([xn3.reshape(n_cores, P, -1), invd, npad,
                           easum.reshape(n_cores, P, -1)], axis=2)

    sched = dict(T=T, D=D, off=off, S=S, NLOC=NLOC, NPAD=NPAD, n_cores=n_cores)
    streams = dict(ea7=ea7s.astype(np.float16), xgv=xgvs.astype(np.float16),
                   tabs=np.ascontiguousarray(tabs))
    unscr = dict(node_of=node_of, valid_loc=orders < NLOC)
    return sched, streams, unscr


def host_weights(n_heads, C, W_gat, att_src, att_dst, W_edge, att_edge,
                 bias_gat, W1, b1, prelu_a, W2, b2):
    """Pure-layout reshapes/replications/casts of the weight tensors.

    packw [P, 20] fp32:  0:3 W_gat.T | 3:10 W_edge.T | 10 att_src |
      11 att_dst | 12 att_edge | 13:17 hmask | 17 bias_gat | 18 b1 |
      19 b2 (rows 0:32)
    pack16 [P, 160] fp16:  0:128 W1 | 128:160 W2
    """
    HC = n_heads * C
    nj_x = W_gat.shape[0]
    packw = np.zeros((P, 20), dtype=np.float32)
    packw[:, 0:3] = W_gat.T
    packw[:, 3:10] = W_edge.T
    packw[:, 10] = att_src.reshape(HC)
    packw[:, 11] = att_dst.reshape(HC)
    packw[:, 12] = att_edge.reshape(HC)
    for h in range(n_heads):
        packw[h * C:(h + 1) * C, 13 + h] = 1.0
    packw[:, 17] = bias_gat
    packw[:, 18] = b1
    packw[:b2.shape[0], 19] = b2
    pack16 = np.zeros((P, HC + 32), dtype=np.float16)
    pack16[:, 0:HC] = W1
    pack16[:, HC:HC + 32] = W2
    wpj = np.zeros((nj_x * n_heads, HC), dtype=np.float32)
    for h in range(n_heads):
        wpj[nj_x * h: nj_x * (h + 1), C * h: C * (h + 1)] = \
            W_gat[:, C * h: C * (h + 1)]
    return dict(
        packw=packw,
        pack16=pack16,
        wpj16=np.ascontiguousarray(wpj, dtype=np.float16),
        ident=np.eye(P, dtype=np.float32),
    )


# ---------------------------------------------------------------------------
# Device program.
# ---------------------------------------------------------------------------
def build_program(sched, n_heads=4, nj_x=3, nj_e=7, lat=32,
                  prelu_alpha=0.25):
    T = sched["T"]
    D = sched["D"]
    off = sched["off"]
    S = sched["S"]
    HC = P  # hidden dim == 128 == partitions
    H = n_heads
    NG = (T + CH - 1) // CH  # chunk groups

    nc = bass.Bass()
    dt = F32

    # --- dram I/O ---
    ea7_d = nc.dram_tensor("ea7", [P, nj_e * S], F16, kind="ExternalInput")
    xgv_d = nc.dram_tensor("xgv", [P, nj_x * S], F16, kind="ExternalInput")
    tabs_d = nc.dram_tensor("tabs", [P, 12 * T], dt, kind="ExternalInput")
    packw_d = nc.dram_tensor("packw", [P, 20], dt, kind="ExternalInput")
    pack16_d = nc.dram_tensor("pack16", [P, HC + lat], F16,
                              kind="ExternalInput")
    wpj16_d = nc.dram_tensor("wpj16", [nj_x * H, HC], F16, kind="ExternalInput")
    id_d = nc.dram_tensor("ident", [P, P], dt, kind="ExternalInput")
    # output in channel-major [lat, (t, p)]; host transposes in unscramble
    out_d = nc.dram_tensor("out", [lat, T * P], dt, kind="ExternalOutput")

    NSC = nj_e * H + nj_x * H + nj_x * H  # scale columns: V | U_src | U_dst
    OFF_V, OFF_US, OFF_UD = 0, nj_e * H, nj_e * H + nj_x * H

    # blocks split at chunk boundaries; small first block to fill the pipe,
    # small last block to shorten the un-overlapped phase-2 tail
    bl = block_split(T)
    SBmax = max(int(off[t1] - off[t0]) for (t0, t1) in bl)
    Dmax = int(D.max())
    CW = CH * P  # phase-2 chunk column width

    with tile.TileContext(nc) as tc:
        with (
            tc.tile_pool(name="wp", bufs=1) as wp,
            tc.tile_pool(name="sp", bufs=2) as sp,
            tc.tile_pool(name="mp", bufs=3) as mp,
            tc.tile_pool(name="pp", bufs=2, space="PSUM") as pp,
            tc.tile_pool(name="pq", bufs=1, space="PSUM") as pq,
        ):
            # ---------------- phase 0: weights & derived ----------------
            pw = wp.tile([P, 20], dt, tag="pw")
            tabs = wp.tile([P, 12 * T], dt, tag="tabs")
            pk16 = wp.tile([P, HC + lat], F16, tag="pk16")
            wpj = wp.tile([nj_x * H, HC], F16, tag="wpj")
            ident = wp.tile([P, P], dt, tag="ident")
            nc.sync.dma_start(pw[:], packw_d[:])
            nc.sync.dma_start(tabs[:], tabs_d[:])
            # slices of the packed tensors
            wgT = pw[:, 0:3]
            weT = pw[:, 3:10]
            asc = pw[:, 10:11]
            adc = pw[:, 11:12]
            aec = pw[:, 12:13]
            bgc = pw[:, 17:18]
            b1c = pw[:, 18:19]
            b2c = pw[0:lat, 19:20]
            w1s = pk16[:, 0:HC]
            w2s = pk16[:, HC:HC + lat]
            XNS0, IVD0, NPD0, EAS0 = 0, 3 * T, 4 * T, 5 * T
            onesr = wp.tile([1, P], dt, tag="onesr")
            nc.vector.memset(onesr[:], 1.0)
            mcol = wp.tile([P, 1], dt, tag="mcol")
            nc.vector.memset(mcol[:], MSHIFT)

            # W28 = W_edgeT (j-major x H) * head-mask ; W12 same from W_gatT
            w28 = wp.tile([HC, nj_e * H], dt, tag="w28")
            w12 = wp.tile([HC, nj_x * H], dt, tag="w12")
            pw_t, pw_o = pw[:].tensor, pw[:].offset
            pw_p = list(pw[:].ap[0])
            weT_b = AP(pw_t, pw_o + 3, [pw_p, [1, nj_e], [0, H]])
            hm_e = AP(pw_t, pw_o + 13, [pw_p, [0, nj_e], [1, H]])
            nc.vector.tensor_tensor(
                out=w28[:].rearrange("p (j h) -> p j h", j=nj_e),
                in0=weT_b, in1=hm_e, op=OP.mult)
            wgT_b = AP(pw_t, pw_o + 0, [pw_p, [1, nj_x], [0, H]])
            hm_x = AP(pw_t, pw_o + 13, [pw_p, [0, nj_x], [1, H]])
            nc.vector.tensor_tensor(
                out=w12[:].rearrange("p (j h) -> p j h", j=nj_x),
                in0=wgT_b, in1=hm_x, op=OP.mult)

            # scale rows via K=128 matmuls, then partition-broadcast
            ps1w = pp.tile([HC, CW], dt, tag="ps1")
            ps2w = pp.tile([HC, CW], dt, tag="ps2")
            srow = wp.tile([1, NSC], dt, tag="srow")
            psv = ps1w[0:1, 0:NSC]
            nc.tensor.matmul(psv[:, 0:nj_e * H], aec, w28[:],
                             start=True, stop=True)
            nc.tensor.matmul(psv[:, OFF_US:OFF_US + nj_x * H], asc, w12[:],
                             start=True, stop=True)
            nc.tensor.matmul(psv[:, OFF_UD:OFF_UD + nj_x * H], adc, w12[:],
                             start=True, stop=True)
            nc.vector.tensor_copy(srow[:], psv)
            scal = wp.tile([P, NSC], dt, tag="scal")
            psb = ps2w[0:P, 0:NSC]
            nc.tensor.matmul(psb, onesr[:], srow[:], start=True, stop=True)
            nc.vector.tensor_copy(scal[:], psb)

            ad_all = wp.tile([P, H * T], dt, tag="ad_all")
            ad16 = wp.tile([P, H * T], F16, tag="ad16")
            corr = wp.tile([P, H * T], dt, tag="corr")
            tpr = wp.tile([P, H * T], F16, tag="tpr")
            av_all = wp.tile([P, H * T], dt, tag="av_all")
            avm16 = wp.tile([P, H * T], F16, tag="avm16")
            LV = wp.tile([P, H * nj_e * SBmax], F16, tag="LV")

            def emit_ad_corr():
                # ad_all [P, H, T] from xn planes (a_dst per node)
                for h in range(H):
                    adh = ad_all[:, h * T:(h + 1) * T]
                    nc.vector.tensor_scalar(
                        out=adh, in0=tabs[:, XNS0:XNS0 + T],
                        scalar1=scal[:, OFF_UD + h: OFF_UD + h + 1],
                        scalar2=None, op0=OP.mult)
                    for j in range(1, nj_x):
                        nc.vector.scalar_tensor_tensor(
                            out=adh,
                            in0=tabs[:, XNS0 + j * T: XNS0 + (j + 1) * T],
                            scalar=scal[:, OFF_UD + j * H + h:
                                        OFF_UD + j * H + h + 1],
                            in1=adh, op0=OP.mult, op1=OP.add)
                nc.vector.tensor_copy(ad16[:], ad_all[:])
                # pad-slot denominator correction: corr = npad*exp(prelu(ad))
                nc.scalar.activation(tpr[:], ad16[:], AF.Prelu,
                                     alpha=NEG_SLOPE)
                nc.scalar.activation(corr[:], tpr[:], AF.Exp,
                                     bias=mcol[:, :1])
                npd_b = AP(tabs[:].tensor, tabs[:].offset + NPD0,
                           [list(tabs[:].ap[0]), [0, H], [1, T]])
                nc.vector.tensor_tensor(out=corr[:].rearrange(
                    "p (h t) -> p h t", h=H), in0=corr[:].rearrange(
                    "p (h t) -> p h t", h=H), in1=npd_b, op=OP.mult)
                # self-loop logit base: mean of incoming V-logits per node
                # av_all[p,h,t] = sum_j easum_j * V[j,h];  avm16 = av * invd
                for h in range(H):
                    avh = av_all[:, h * T:(h + 1) * T]
                    nc.vector.tensor_scalar(
                        out=avh, in0=tabs[:, EAS0:EAS0 + T],
                        scalar1=scal[:, OFF_V + h: OFF_V + h + 1],
                        scalar2=None, op0=OP.mult)
                    for j in range(1, nj_e):
                        nc.vector.scalar_tensor_tensor(
                            out=avh,
                            in0=tabs[:, EAS0 + j * T: EAS0 + (j + 1) * T],
                            scalar=scal[:, OFF_V + j * H + h:
                                        OFF_V + j * H + h + 1],
                            in1=avh, op0=OP.mult, op1=OP.add)
                ivd_f = AP(tabs[:].tensor, tabs[:].offset + IVD0,
                           [list(tabs[:].ap[0]), [0, H], [1, T]])
                nc.vector.tensor_tensor(out=avm16[:].rearrange(
                    "p (h t) -> p h t", h=H), in0=av_all[:].rearrange(
                    "p (h t) -> p h t", h=H), in1=ivd_f, op=OP.mult)

            # persistent per-chunk accumulators
            den_g = [wp.tile([P, H * CH], dt, tag=f"den{g}", name=f"den{g}")
                     for g in range(NG)]
            agg_g = [wp.tile([P, H * nj_x * CH], dt, tag=f"agg{g}",
                             name=f"agg{g}") for g in range(NG)]

            def chunks_of(bt0, bt1):
                return range(bt0 // CH, (bt1 + CH - 1) // CH)

            # ------------- per-block edge pipeline (sw-pipelined) ----------
            # Emission order: trees(0) | [logits-tail(b) ; trees(b+1) ;
            # post-exp(b) ; phase2(b)] so DVE never waits on pool/ACT.
            blk = {}

            def emit_dma(b):
                bt0, bt1 = bl[b]
                o0, o1 = int(off[bt0]), int(off[bt1])
                SB = o1 - o0
                # block-contiguous dram offsets
                eoff = sum(nj_e * (int(off[t1]) - int(off[t0]))
                           for (t0, t1) in bl[:b])
                xoff = sum(nj_x * (int(off[t1]) - int(off[t0]))
                           for (t0, t1) in bl[:b])
                eab = sp.tile([P, nj_e * SBmax], F16, tag="eab")
                xgb = sp.tile([P, nj_x * SBmax], F16, tag="xgb")
                aev = sp.tile([P, H * SBmax], F16, tag="aev")
                exb = sp.tile([P, H * SBmax], F16, tag="exb")
                scu = sp.tile([P, H * nj_x * SBmax], F16, tag="scu")
                blk[b] = (o0, SB, eab, xgb, aev, exb, None, scu)
                half = (nj_e * SB) // 2
                nc.sync.dma_start(eab[:, :half], ea7_d[:, eoff: eoff + half])
                nc.sync.dma_start(eab[:, half: nj_e * SB],
                                  ea7_d[:, eoff + half: eoff + nj_e * SB])
                nc.sync.dma_start(xgb[:, :nj_x * SB],
                                  xgv_d[:, xoff: xoff + nj_x * SB])

            def emit_trees(b):
                bt0, bt1 = bl[b]
                o0, SB, eab, xgb, aev, exb, scr, scu = blk[b]
                ae_t, ae_o = aev[:].tensor, aev[:].offset
                ae_p = list(aev[:].ap[0])
                M = SBmax  # uniform plane stride in LV / scu
                lv_t, lv_o = LV[:].tensor, LV[:].offset
                lv_p = list(LV[:].ap[0])
                su_t, su_o = scu[:].tensor, scu[:].offset
                su_p = list(scu[:].ap[0])
                lvs = lambda k: LV[:, k * M: k * M + SB]
                sus = lambda k: scu[:, k * M: k * M + SB]

                # U-tree leaves + V-leaves j=5,6 on ACT — overlap DVE leaves
                for h in range(H):
                    for j in range(nj_x):
                        nc.scalar.activation(
                            sus(h * nj_x + j), xgb[:, j * SB:(j + 1) * SB],
                            AF.Copy,
                            scale=scal[:, OFF_US + j * H + h:
                                       OFF_US + j * H + h + 1])
                for h in range(H):
                    for j in (5, 6):
                        nc.scalar.activation(
                            lvs(h * nj_e + j), eab[:, j * SB:(j + 1) * SB],
                            AF.Copy,
                            scale=scal[:, OFF_V + j * H + h:
                                       OFF_V + j * H + h + 1])

                # V-leaves j=0..4 on DVE
                for h in range(H):
                    for j in range(5):
                        nc.vector.tensor_scalar(
                            out=lvs(h * nj_e + j),
                            in0=eab[:, j * SB:(j + 1) * SB],
                            scalar1=scal[:, OFF_V + j * H + h:
                                         OFF_V + j * H + h + 1],
                            scalar2=None, op0=OP.mult)

                # merge tree batched across heads (7 TT instrs per block)
                ap3 = lambda t, o, p, d: AP(t, o, [p] + d)
                # B1: lv[7h+{0,2,4}] += lv[7h+{1,3,5}]
                nc.vector.tensor_tensor(
                    out=ap3(lv_t, lv_o, lv_p,
                            [[nj_e * M, H], [2 * M, 3], [1, SB]]),
                    in0=ap3(lv_t, lv_o, lv_p,
                            [[nj_e * M, H], [2 * M, 3], [1, SB]]),
                    in1=ap3(lv_t, lv_o + M, lv_p,
                            [[nj_e * M, H], [2 * M, 3], [1, SB]]),
                    op=OP.add)
                # B2: lv[7h+4] += lv[7h+6]
                nc.vector.tensor_tensor(
                    out=ap3(lv_t, lv_o + 4 * M, lv_p,
                            [[nj_e * M, H], [1, SB]]),
                    in0=ap3(lv_t, lv_o + 4 * M, lv_p,
                            [[nj_e * M, H], [1, SB]]),
                    in1=ap3(lv_t, lv_o + 6 * M, lv_p,
                            [[nj_e * M, H], [1, SB]]),
                    op=OP.add)
                # B3: lv[7h] += lv[7h+2] ; B4: lv[7h] += lv[7h+4]
                for o1 in (2 * M, 4 * M):
                    nc.vector.tensor_tensor(
                        out=ap3(lv_t, lv_o, lv_p, [[nj_e * M, H], [1, SB]]),
                        in0=ap3(lv_t, lv_o, lv_p, [[nj_e * M, H], [1, SB]]),
                        in1=ap3(lv_t, lv_o + o1, lv_p,
                                [[nj_e * M, H], [1, SB]]),
                        op=OP.add)
                # B5/B6: su[3h] += su[3h+1] ; su[3h] += su[3h+2]
                for o1 in (M, 2 * M):
                    nc.vector.tensor_tensor(
                        out=ap3(su_t, su_o, su_p, [[nj_x * M, H], [1, SB]]),
                        in0=ap3(su_t, su_o, su_p, [[nj_x * M, H], [1, SB]]),
                        in1=ap3(su_t, su_o + o1, su_p,
                                [[nj_x * M, H], [1, SB]]),
                        op=OP.add)
                # B7: ae[h] = lv[7h] + su[3h]
                nc.vector.tensor_tensor(
                    out=ap3(ae_t, ae_o, ae_p, [[SB, H], [1, SB]]),
                    in0=ap3(lv_t, lv_o, lv_p, [[nj_e * M, H], [1, SB]]),
                    in1=ap3(su_t, su_o, su_p, [[nj_x * M, H], [1, SB]]),
                    op=OP.add)

                # self-loop slot0 += mean of incoming V-logits (per chunk)
                for g in chunks_of(bt0, bt1):
                    ta, tb = g * CH, min((g + 1) * CH, T)
                    tcn = tb - ta
                    dt_g = int(D[ta])
                    lt = int(off[ta]) - o0
                    sl0 = AP(ae_t, ae_o + lt,
                             [ae_p, [dt_g, tcn], [SB, H]])
                    avm_b = AP(avm16[:].tensor, avm16[:].offset + ta,
                               [list(avm16[:].ap[0]), [1, tcn], [T, H]])
                    nc.vector.tensor_tensor(out=sl0, in0=sl0, in1=avm_b,
                                            op=OP.add)

            def emit_logits_tail(b):
                """a_dst add (GpSimd) + leaky-relu + exp (ACT)."""
                bt0, bt1 = bl[b]
                o0, SB, eab, xgb, aev, exb, scr, scu = blk[b]
                ae_t, ae_o = aev[:].tensor, aev[:].offset
                ae_p = list(aev[:].ap[0])
                for g in chunks_of(bt0, bt1):
                    ta, tb = g * CH, min((g + 1) * CH, T)
                    tcn = tb - ta
                    dt_g = int(D[ta])
                    lt = int(off[ta]) - o0
                    sl = AP(ae_t, ae_o + lt,
                            [ae_p, [dt_g, tcn], [SB, H], [1, dt_g]])
                    adb = AP(ad16[:].tensor, ad16[:].offset + ta,
                             [list(ad16[:].ap[0]), [1, tcn], [T, H], [0, dt_g]])
                    nc.vector.tensor_tensor(out=sl, in0=sl, in1=adb, op=OP.add)
                nc.scalar.activation(aev[:, :H * SB], aev[:, :H * SB],
                                     AF.Prelu, alpha=NEG_SLOPE)
                nc.scalar.activation(exb[:, :H * SB], aev[:, :H * SB], AF.Exp,
                                     bias=mcol[:, :1])

            def emit_post_chunk(b, g):
                """denominators, weighted messages, aggregation for chunk g."""
                o0, SB, eab, xgb, aev, exb, scr, scu = blk[b]
                ex_t, ex_o = exb[:].tensor, exb[:].offset
                ex_p = list(exb[:].ap[0])
                if True:
                    ta, tb = g * CH, min((g + 1) * CH, T)
                    tcn = tb - ta
                    dt_g = int(D[ta])
                    lt = int(off[ta]) - o0
                    dg = den_g[g]
                    ag = agg_g[g]

                    nc.vector.tensor_reduce(
                        out=AP(dg[:].tensor, dg[:].offset,
                               [list(dg[:].ap[0]), [1, tcn], [CH, H]]),
                        in_=AP(ex_t, ex_o + lt,
                               [ex_p, [dt_g, tcn], [SB, H], [1, dt_g]]),
                        axis=mybir.AxisListType.X, op=OP.add)
                    # den -= npad * exp(prelu(a_dst))   (pad-slot correction)
                    cor_b = AP(corr[:].tensor, corr[:].offset + ta,
                               [list(corr[:].ap[0]), [1, tcn], [T, H]])
                    dg_b = AP(dg[:].tensor, dg[:].offset,
                              [list(dg[:].ap[0]), [1, tcn], [CH, H]])
                    nc.vector.tensor_tensor(out=dg_b, in0=dg_b, in1=cor_b,
                                            op=OP.subtract)

                    msg = mp.tile([P, H * nj_x * CH * Dmax], F16, tag="msg")
                    m_t, m_o, m_p = (msg[:].tensor, msg[:].offset,
                                     list(msg[:].ap[0]))
                    nd = nj_x * dt_g
                    for ti in range(ta, tb):
                        lt_i = int(off[ta]) - o0 + (ti - ta) * dt_g
                        m_ap = AP(m_t, m_o + (ti - ta) * H * nd,
                                  [m_p, [nd, H], [dt_g, nj_x], [1, dt_g]])
                        ealpha = AP(ex_t, ex_o + lt_i,
                                    [ex_p, [SB, H], [0, nj_x], [1, dt_g]])
                        xgs = AP(xgb[:].tensor, xgb[:].offset + lt_i,
                                 [list(xgb[:].ap[0]), [0, H], [SB, nj_x],
                                  [1, dt_g]])
                        nc.vector.tensor_tensor(out=m_ap, in0=ealpha, in1=xgs,
                                                op=OP.mult)
                    nc.vector.tensor_reduce(
                        out=AP(ag[:].tensor, ag[:].offset,
                               [list(ag[:].ap[0]), [1, tcn], [CH, H * nj_x]]),
                        in_=AP(m_t, m_o,
                               [m_p, [H * nd, tcn], [dt_g, H * nj_x],
                                [1, dt_g]]),
                        axis=mybir.AxisListType.X, op=OP.add)

            def emit_phase2_chunk(g, tail=False):
                if True:
                    ta, tb = g * CH, min((g + 1) * CH, T)
                    tcn = tb - ta
                    cw = tcn * P
                    dg = den_g[g]
                    ag = agg_g[g]

                    # alpha-normalize: agg_n = agg / den (fp32)
                    rec = mp.tile([P, H * CH], dt, tag="rec")
                    nc.vector.reciprocal(rec[:], dg[:])
                    agn = mp.tile([P, H * nj_x * CH], dt, tag="agn")
                    ag_b = AP(ag[:].tensor, ag[:].offset,
                              [list(ag[:].ap[0]), [CH * nj_x, H], [CH, nj_x],
                               [1, tcn]])
                    an_b = AP(agn[:].tensor, agn[:].offset,
                              [list(agn[:].ap[0]), [CH * nj_x, H], [CH, nj_x],
                               [1, tcn]])
                    rec_b = AP(rec[:].tensor, rec[:].offset,
                               [list(rec[:].ap[0]), [CH, H], [0, nj_x],
                                [1, tcn]])
                    nc.vector.tensor_tensor(out=an_b, in0=ag_b, in1=rec_b,
                                            op=OP.mult)

                    # transpose agg_n -> [12, cw] then to fp16 for matmul
                    pst = pq.tile([nj_x * H, CW], dt, tag="pst")
                    for ti in range(ta, tb):
                        nc.tensor.transpose(
                            out=pst[:, (ti - ta) * P:(ti - ta + 1) * P],
                            in_=AP(agn[:].tensor, agn[:].offset + (ti - ta),
                                   [list(agn[:].ap[0]), [CH, nj_x * H]]),
                            identity=ident[:])
                    aggT = mp.tile([nj_x * H, CW], F16, tag="aggT")
                    if tail:
                        nc.vector.tensor_copy(aggT[:, :cw], pst[:, :cw])
                    else:
                        nc.scalar.copy(aggT[:, :cw], pst[:, :cw])

                    ps1 = pp.tile([HC, CW], dt, tag="ps1")
                    nc.tensor.matmul(ps1[:, :cw], wpj[:], aggT[:, :cw],
                                     start=True, stop=True)
                    # ELU(z+bg): min(exp(z+bg),1) - 1 + relu(z+bg)
                    r1 = mp.tile([HC, CW], F16, tag="r1")
                    u1 = mp.tile([HC, CW], F16, tag="u1")
                    nc.scalar.activation(r1[:, :cw], ps1[:, :cw], AF.Relu,
                                         bias=bgc)
                    nc.scalar.activation(u1[:, :cw], ps1[:, :cw], AF.Exp,
                                         bias=bgc)
                    nc.vector.tensor_scalar(out=u1[:, :cw], in0=u1[:, :cw],
                                            scalar1=1.0, scalar2=-1.0,
                                            op0=OP.min, op1=OP.add)
                    nc.vector.tensor_tensor(out=r1[:, :cw], in0=r1[:, :cw],
                                            in1=u1[:, :cw], op=OP.add)

                    ps2 = pp.tile([HC, CW], dt, tag="ps2")
                    nc.tensor.matmul(ps2[:, :cw], w1s[:], r1[:, :cw],
                                     start=True, stop=True)
                    h2 = mp.tile([HC, CW], F16, tag="h2")
                    nc.scalar.activation(h2[:, :cw], ps2[:, :cw], AF.Prelu,
                                         bias=b1c, alpha=prelu_alpha)

                    ps3 = pp.tile([lat, CW], dt, tag="ps3")
                    nc.tensor.matmul(ps3[:, :cw], w2s[:], h2[:, :cw],
                                     start=True, stop=True)
                    o3 = mp.tile([lat, CW], dt, tag="o3")
                    if tail:
                        nc.vector.tensor_scalar(
                            out=o3[:, :cw], in0=ps3[:, :cw],
                            scalar1=b2c, scalar2=None, op0=OP.add)
                    else:
                        nc.scalar.activation(o3[:, :cw], ps3[:, :cw],
                                             AF.Identity, bias=b2c)
                    nc.sync.dma_start(out_d[:, ta * P: ta * P + cw],
                                      o3[:, :cw])

            NB = len(bl)
            for b in range(min(2, NB)):
                emit_dma(b)
            # phase-2-only weights: issue after the first input blocks
            nc.sync.dma_start(pk16[:], pack16_d[:])
            nc.sync.dma_start(wpj[:], wpj16_d[:])
            nc.sync.dma_start(ident[:], id_d[:])
            emit_ad_corr()
            emit_trees(0)
            p2q = []  # chunks with post emitted, phase2 pending (lag 2)
            for b in range(NB):
                emit_logits_tail(b)
                if b + 2 < NB:
                    emit_dma(b + 2)
                if b + 1 < NB:
                    emit_trees(b + 1)
                for g in chunks_of(*bl[b]):
                    emit_post_chunk(b, g)
                    p2q.append(g)
                    if len(p2q) > 2:
                        emit_phase2_chunk(p2q.pop(0), tail=(b == NB - 1))
            for g in p2q:
                emit_phase2_chunk(g, tail=True)

    return nc


# ---------------------------------------------------------------------------
# Full kernel entry (host orchestration).
# ---------------------------------------------------------------------------
def make_in_maps(sched, streams, w, n_cores):
    maps = []
    for c in range(n_cores):
        m = dict(
            ea7=streams["ea7"][c].reshape(P, -1),
            xgv=streams["xgv"][c].reshape(P, -1),
            tabs=streams["tabs"][c],
        )
        m.update(w)
        maps.append(m)
    return maps


def unscramble(results, sched, unscr, N, lat=32):
    n_cores = sched["n_cores"]
    T = sched["T"]
    out = np.zeros((N, lat), dtype=np.float32)
    for c in range(n_cores):
        o = results[c]["out"].reshape(lat, T, P).transpose(2, 1, 0)
        node_of = unscr["node_of"][c]  # [T, P] global ids (clamped for dummies)
        valid = unscr["valid_loc"][c].reshape(T, P)
        for t in range(T):
            v = valid[t]
            out[node_of[t][v]] = o[v, t]
    return out


# ---------------------------------------------------------------------------
# Self-contained harness entry: kernel(**inputs) -> full [N, 32] output.
# ---------------------------------------------------------------------------
_CACHE = {}


def kernel(x, edge_index, edge_attr, W_gat, att_src, att_dst, W_edge,
           att_edge, bias_gat, W1, b1, prelu_a, W2, b2):
    from concourse.bass_utils import run_bass_kernel_spmd

    patch_tile_epilogue()
    n_cores = 8
    x = np.asarray(x)
    edge_index = np.asarray(edge_index)
    edge_attr = np.asarray(edge_attr)
    H, C = np.asarray(att_src).shape

    sched, streams, unscr = host_prep(x, edge_index, edge_attr, n_cores)
    w = host_weights(H, C, np.asarray(W_gat), np.asarray(att_src),
                     np.asarray(att_dst), np.asarray(W_edge),
                     np.asarray(att_edge), np.asarray(bias_gat),
                     np.asarray(W1), np.asarray(b1), np.asarray(prelu_a),
                     np.asarray(W2), np.asarray(b2))

    key = (sched["T"], sched["S"], tuple(int(d) for d in sched["D"]),
           float(np.asarray(prelu_a)))
    if key not in _CACHE:
        _CACHE[key] = build_program(sched, n_heads=H,
                                    prelu_alpha=float(np.asarray(prelu_a)))
    nc = _CACHE[key]

    maps = make_in_maps(sched, streams, w, n_cores)
    res = run_bass_kernel_spmd(nc, maps, core_ids=list(range(n_cores)))
    out = unscramble(res.results, sched, unscr, x.shape[0])
    return out.astype(np.float32)
